# revision 1
# baseline (speedup 1.0000x reference)
"""Trainium2 Bass kernel for nn_BasicBlock (spiking CNN block).

Sharding: data-parallel over batch B across 8 NeuronCores (4 batch x 4
timesteps = 16 images per core); BN batch stats via tiny AllReduce.

Per core:
- conv1: 3x3 taps as TensorEngine matmuls in fp16 hi/lo split arithmetic
  (~fp32 accuracy at bf16 speed): per tap [W1hi;W1hi] x [xhi;xlo] (K=128)
  + W1lo x xhi (K=64). Two images run concurrently via PE column tiling.
- BN stats (sum / sum-of-squares) accumulated during PSUM evacuation
  (ScalarE copy w/ accum_out; DVE square pass), all-reduced across cores.
- PLIF scan in "q-space" (conv-output units): BN scale/bias folded into
  per-channel threshold theta / constants, so no per-element BN apply.
- conv2 consumes exact 0/1 spikes in fp16: per tap [W2hi;W2lo] x [s1;s1]
  (K=128) gives both split terms in one matmul.
- Residual + LIF2 streamed in half-strips; out written via casting DMA.
"""
import sys
sys.path.insert(0, '/opt/trn_rl_repo')

import numpy as np

T, B, C, H, W = 4, 32, 64, 56, 56
NCORES = 8
BL = B // NCORES            # 4 local batch samples
NIMG = T * BL               # 16 images per core
HP = W + 2                  # 58
PP = HP * HP                # 3364 padded pixels
PIX = H * W                 # 3136
NCH = 7                     # conv chunks per image (8 rows each)
CHW = 8 * W                 # 448
NPAIR = 8                   # image pairs per core
EPS = 1e-5
NG = float((T * B) * PIX)   # 401408
QL = 14 * W                 # LIF quarter-strip length (784)
NQ = 4

_prog_cache = {}
DBG = False
NO_CC = False
PHASES = 3
TRACE = False
LAST_RES = None
LAST_NAMES = None
LAST_EXEC_NS = None


def _build(alpha1, alpha2):
    import concourse.mybir as mybir
    import concourse.tile as tile
    from concourse import bacc

    F32 = mybir.dt.float32
    F16 = mybir.dt.float16
    AO = mybir.AluOpType
    AF = mybir.ActivationFunctionType
    AX = mybir.AxisListType

    nc = bacc.Bacc(None, target_bir_lowering=False)
    names = {}

    with tile.TileContext(nc) as tc:
        with tc.tile_pool(name="dram", bufs=1, space="DRAM") as dram:
            xta = dram.tile([NIMG, 2, 64, PP], F16, kind="ExternalInput")
            xin = dram.tile([NIMG, 64, PIX], F32, kind="ExternalInput")
            w1a = dram.tile([128, 9 * 64], F16, kind="ExternalInput")
            w1b = dram.tile([128, 9 * 64], F16, kind="ExternalInput")
            w2a = dram.tile([128, 9 * 64], F16, kind="ExternalInput")
            cpar = dram.tile([128, 8], F32, kind="ExternalInput")
            outp = dram.tile([NIMG, 64, PIX], F32, kind="ExternalOutput")
            names.update(xta=xta.name, xin=xin.name, w1a=w1a.name,
                         w1b=w1b.name, w2a=w2a.name, cpar=cpar.name,
                         outp=outp.name)
            if DBG:
                y1d = dram.tile([NPAIR, 128, PIX], F32, kind="ExternalOutput")
                y2d = dram.tile([NPAIR, 128, PIX], F32, kind="ExternalOutput")
                s1d = dram.tile([NPAIR, 128, PIX], F32, kind="ExternalOutput")
                vecd = dram.tile([128, 8], F32, kind="ExternalOutput")
                names.update(y1d=y1d.name, y2d=y2d.name, s1d=s1d.name,
                             vecd=vecd.name)

            with tc.tile_pool(name="dramw", bufs=1, space="DRAM") as dramw, \
                 tc.tile_pool(name="wsb", bufs=1) as wsb, \
                 tc.tile_pool(name="ys", bufs=8) as yspool, \
                 tc.tile_pool(name="plane", bufs=4) as plpool, \
                 tc.tile_pool(name="hfp", bufs=2) as hf, \
                 tc.tile_pool(name="tiny", bufs=40) as tiny, \
                 tc.tile_pool(name="ps", bufs=7, space="PSUM") as ps:

                # ---- static parameter loads
                w1as = wsb.tile([128, 9 * 64], F16, tag="w1a")
                nc.sync.dma_start(w1as[:], w1a[:])
                w1bs = wsb.tile([128, 9 * 64], F16, tag="w1b")
                nc.sync.dma_start(w1bs[:], w1b[:])
                w2as = wsb.tile([128, 9 * 64], F16, tag="w2a")
                nc.sync.dma_start(w2as[:], w2a[:])
                cpars = wsb.tile([128, 8], F32, tag="cpar")
                nc.sync.dma_start(cpars[:], cpar[:])
                sums1 = wsb.tile([128, 56], F32, tag="sums1")
                sums1q = wsb.tile([128, 56], F32, tag="sums1q")
                sums2 = wsb.tile([128, 56], F32, tag="sums2")
                sums2q = wsb.tile([128, 56], F32, tag="sums2q")
                if PHASES < 2:
                    nc.vector.memset(sums2[:], 0.0)
                    nc.vector.memset(sums2q[:], 0.0)

                def conv_img_pair(plA, plB, lhi, llo, dst_strip, sums_t,
                                  sumsq_t, pcol):
                    """One image pair -> 7 chunks in two waves (4+3); taps
                    outer within a wave so consecutive matmuls hit different
                    PSUM banks and weight loads amortize; wave evacuations
                    overlap the next wave's matmuls."""
                    plAr = plA.rearrange("p (r w) -> p r w", w=HP)
                    plBr = plB.rearrange("p (r w) -> p r w", w=HP)
                    for wave in (range(0, 4), range(4, 7)):
                        pts = {}
                        for cth in wave:
                            pts[cth] = ps.tile([128, CHW], F32, tag="ps",
                                               bufs=7, name=f"psum{cth}")
                        last_a = 8 if llo is None else None
                        for a in range(9):
                            di, dj = a // 3, a % 3
                            for cth in wave:
                                r0 = 8 * cth + di
                                for j, plr in enumerate((plAr, plBr)):
                                    rhs = plr[:, r0:r0 + 8, dj:dj + W]
                                    out = pts[cth][64 * j:64 * (j + 1), :] \
                                        .rearrange("p (r w) -> p r w", r=8)
                                    nc.tensor.matmul(
                                        out, lhi[:, a * 64:(a + 1) * 64], rhs,
                                        start=(a == 0), stop=(a == last_a),
                                        tile_position=(0, 64 * j),
                                        skip_group_check=True)
                        if llo is not None:
                            for a in range(9):
                                di, dj = a // 3, a % 3
                                for cth in wave:
                                    r0 = 8 * cth + di
                                    for j, plr in enumerate((plAr, plBr)):
                                        rhs = plr[:, r0:r0 + 8, dj:dj + W]
                                        out = pts[cth][64 * j:64 * (j + 1), :] \
                                            .rearrange("p (r w) -> p r w", r=8)
                                        nc.tensor.matmul(
                                            out, llo[:, a * 64:(a + 1) * 64], rhs,
                                            start=False, stop=(a == 8),
                                            tile_position=(0, 64 * j),
                                            skip_group_check=True)
                        for cth in wave:
                            nc.scalar.activation(
                                dst_strip[:, CHW * cth:CHW * (cth + 1)],
                                pts[cth][:], AF.Copy,
                                accum_out=sums_t[:, pcol * 7 + cth:pcol * 7 + cth + 1])
                            jk = ps.tile([128, CHW], F32, tag="psjk", bufs=1,
                                         name="psjk")
                            sl = dst_strip[:, CHW * cth:CHW * (cth + 1)]
                            nc.vector.scalar_tensor_tensor(
                                jk[:], sl, 1.0, sl, AO.bypass, AO.mult,
                                accum_out=sumsq_t[:, pcol * 7 + cth:pcol * 7 + cth + 1])

                # ================= phase A: conv1 =================
                y1s = []
                for p in range(NPAIR):
                    tt_, bp = p // 2, p % 2
                    iA = tt_ * 4 + bp * 2
                    planes = []
                    for j in range(2):
                        i = iA + j
                        ta = plpool.tile([128, PP], F16, tag="ta")
                        nc.sync.dma_start(ta[0:64, :], xta[i, 0])
                        nc.sync.dma_start(ta[64:128, :], xta[i, 1])
                        planes.append(ta)
                    strip = yspool.tile([128, PIX], F32, tag="ys")
                    y1s.append(strip)
                    conv_img_pair(planes[0], planes[1], w1as, w1bs, strip,
                                  sums1, sums1q, p)
                    if DBG:
                        nc.sync.dma_start(y1d[p], strip[:])

                # ---- stats1 allreduce
                cc1i = dramw.tile([128, 2], F32)
                cc1o = dramw.tile([128, 2], F32, addr_space="Shared")
                acc1 = tiny.tile([128, 2], F32, tag="acc")
                nc.vector.tensor_reduce(acc1[:, 0:1], sums1[:], AX.X, AO.add)
                nc.vector.tensor_reduce(acc1[:, 1:2], sums1q[:], AX.X, AO.add)
                nc.sync.dma_start(cc1i[:], acc1[:])
                if NO_CC:
                    nc.sync.dma_start(cc1o[:], cc1i[:])
                else:
                    nc.gpsimd.collective_compute(
                        "AllReduce", AO.add, ins=[cc1i[:]], outs=[cc1o[:]],
                        replica_groups=[list(range(NCORES))])
                g1 = tiny.tile([128, 2], F32, tag="acc")
                nc.sync.dma_start(g1[:], cc1o[:])

                shuf_mask = [(i + 16) % 32 for i in range(32)]

                def stats_block(g, gamma, beta, rga, rgam, alpha):
                    gr = tiny.tile([128, 2], F32, tag="acc")
                    nc.sync.dma_start(gr[0:64, :], g[64:128, :])
                    nc.sync.dma_start(gr[64:128, :], g[0:64, :])
                    tot = tiny.tile([128, 2], F32, tag="acc")
                    nc.vector.tensor_tensor(tot[:], g[:], gr[:], AO.add)
                    mean = tiny.tile([128, 1], F32, tag="t1")
                    nc.vector.tensor_scalar(mean[:], tot[:, 0:1], 1.0 / NG,
                                            None, AO.mult)
                    msq = tiny.tile([128, 1], F32, tag="t1")
                    nc.vector.tensor_scalar(msq[:], tot[:, 1:2], 1.0 / NG,
                                            None, AO.mult)
                    m2 = tiny.tile([128, 1], F32, tag="t1")
                    nc.vector.scalar_tensor_tensor(m2[:], mean[:], 1.0, mean[:],
                                                   AO.bypass, AO.mult)
                    var = tiny.tile([128, 1], F32, tag="t1")
                    nc.vector.tensor_tensor(var[:], msq[:], m2[:], AO.subtract)
                    epst = tiny.tile([128, 1], F32, tag="t1")
                    nc.vector.memset(epst[:], EPS)
                    std = tiny.tile([128, 1], F32, tag="t1")
                    nc.scalar.activation(std[:], var[:], AF.Sqrt, bias=epst[:])
                    rstd = tiny.tile([128, 1], F32, tag="t1")
                    nc.vector.reciprocal(rstd[:], std[:])
                    sc = tiny.tile([128, 1], F32, tag="t1")
                    nc.vector.tensor_tensor(sc[:], gamma, rstd[:], AO.mult)
                    nmsc = tiny.tile([128, 1], F32, tag="t1")
                    nc.vector.scalar_tensor_tensor(nmsc[:], mean[:], -1.0, sc[:],
                                                   AO.mult, AO.mult)
                    bi = tiny.tile([128, 1], F32, tag="t1")
                    nc.vector.tensor_tensor(bi[:], beta, nmsc[:], AO.add)
                    stdrg = tiny.tile([128, 1], F32, tag="t1")
                    nc.vector.tensor_tensor(stdrg[:], std[:], rga, AO.mult)
                    nbst = tiny.tile([128, 1], F32, tag="t1")
                    nc.vector.scalar_tensor_tensor(nbst[:], bi[:], -alpha,
                                                   stdrg[:], AO.mult, AO.mult)
                    th = tiny.tile([128, 1], F32, tag="t1")
                    nc.vector.tensor_tensor(th[:], stdrg[:], nbst[:], AO.add)
                    bstd = tiny.tile([128, 1], F32, tag="t1")
                    nc.vector.tensor_tensor(bstd[:], bi[:], std[:], AO.mult)
                    gamv = tiny.tile([128, 1], F32, tag="t1")
                    nc.vector.tensor_tensor(gamv[:], bstd[:], rgam, AO.mult)
                    rscv = tiny.tile([128, 1], F32, tag="t1")
                    nc.vector.tensor_tensor(rscv[:], std[:], rgam, AO.mult)
                    gmw = tiny.tile([128, 1], F32, tag="t1")
                    nc.vector.tensor_scalar(gmw[:], gamv[:], 1.0 - alpha, None,
                                            AO.mult)
                    return th, gamv, rscv, gmw

                th1, gm1, _rsc1, gmw1 = stats_block(
                    g1, cpars[:, 0:1], cpars[:, 1:2], cpars[:, 4:5],
                    cpars[:, 6:7], alpha1)
                if DBG:
                    nc.sync.dma_start(vecd[:, 0:1], th1[:])
                    nc.sync.dma_start(vecd[:, 1:2], gm1[:])
                    nc.sync.dma_start(vecd[:, 4:5], acc1[:, 0:1])
                    nc.sync.dma_start(vecd[:, 5:6], acc1[:, 1:2])

                # ============ phase B + C: LIF1 + conv2 ============
                y2s = [None] * NPAIR
                for bp in range(2 if PHASES >= 2 else 0):
                    Pprev = [None] * NQ
                    for t in range(1, 5):
                        p = (t - 1) * 2 + bp
                        s1tq = []
                        for hq in range(NQ):
                            off = QL * hq
                            ysl = y1s[p][:, off:off + QL]
                            if t == 1:
                                qa = ysl
                            else:
                                q = hf.tile([128, QL], F32, tag="q2", bufs=4)
                                nc.gpsimd.tensor_tensor(q[:], ysl,
                                                        Pprev[hq][:], AO.add)
                                qa = q[:]
                            s1t = hf.tile([128, QL], F16, tag="s1t", bufs=4)
                            nc.vector.tensor_scalar(s1t[:], qa, th1[:],
                                                    None, AO.is_ge)
                            s1tq.append(s1t)
                            if DBG:
                                nc.gpsimd.dma_start(
                                    s1d[p, :, off:off + QL], s1t[:])
                            if t < 4:
                                sb = hf.tile([128, QL], F16, tag="sb", bufs=2)
                                nc.vector.tensor_scalar(sb[:], qa, th1[:],
                                                        None, AO.is_lt)
                                wv = hf.tile([128, QL], F32, tag="q2", bufs=4)
                                nc.vector.tensor_scalar(
                                    wv[:], qa, gm1[:], 1.0 - alpha1,
                                    AO.add, AO.mult)
                                Pn = hf.tile([128, QL], F32, tag="pp", bufs=6)
                                nc.vector.tensor_tensor(Pn[:], wv[:], sb[:],
                                                        AO.mult)
                                Pprev[hq] = Pn
                        iA = (t - 1) * 4 + bp * 2
                        tas_pair = []
                        for j in range(2):
                            tas = plpool.tile([128, PP], F16, tag="ta")
                            tasr = tas.rearrange("p (r w) -> p r w", w=HP)
                            nc.gpsimd.memset(tas[:, 0:HP], 0.0)
                            nc.gpsimd.memset(tas[:, PP - HP:PP], 0.0)
                            nc.gpsimd.memset(tasr[:, :, 0:1], 0.0)
                            nc.gpsimd.memset(tasr[:, :, HP - 1:HP], 0.0)
                            for hq in range(NQ):
                                src = s1tq[hq][64 * j:64 * (j + 1), :] \
                                    .rearrange("p (r w) -> p r w", w=W)
                                dsti = tasr[:, 1 + 14 * hq:1 + 14 * (hq + 1),
                                            1:1 + W]
                                nc.sync.dma_start(dsti[0:64], src)
                                nc.sync.dma_start(dsti[64:128], src)
                            tas_pair.append(tas)
                        strip2 = yspool.tile([128, PIX], F32, tag="ys")
                        y2s[p] = strip2
                        conv_img_pair(tas_pair[0], tas_pair[1], w2as, None,
                                      strip2, sums2, sums2q, p)
                        if DBG:
                            nc.sync.dma_start(y2d[p], strip2[:])

                # ---- stats2 allreduce
                cc2i = dramw.tile([128, 2], F32)
                cc2o = dramw.tile([128, 2], F32, addr_space="Shared")
                acc2 = tiny.tile([128, 2], F32, tag="acc")
                nc.vector.tensor_reduce(acc2[:, 0:1], sums2[:], AX.X, AO.add)
                nc.vector.tensor_reduce(acc2[:, 1:2], sums2q[:], AX.X, AO.add)
                nc.sync.dma_start(cc2i[:], acc2[:])
                if NO_CC:
                    nc.sync.dma_start(cc2o[:], cc2i[:])
                else:
                    nc.gpsimd.collective_compute(
                        "AllReduce", AO.add, ins=[cc2i[:]], outs=[cc2o[:]],
                        replica_groups=[list(range(NCORES))])
                g2 = tiny.tile([128, 2], F32, tag="acc")
                nc.sync.dma_start(g2[:], cc2o[:])
                th2, gm2, rsc2, gmw2 = stats_block(
                    g2, cpars[:, 2:3], cpars[:, 3:4], cpars[:, 5:6],
                    cpars[:, 7:8], alpha2)
                if DBG:
                    nc.sync.dma_start(vecd[:, 2:3], th2[:])
                    nc.sync.dma_start(vecd[:, 3:4], gm2[:])

                # ============ phase D: residual + LIF2 ============
                for bp in range(2 if PHASES >= 3 else 0):
                    Pprev2 = [None] * NQ
                    for t in range(1, 5):
                        p = (t - 1) * 2 + bp
                        iA = (t - 1) * 4 + bp * 2
                        for hq in range(NQ):
                            off = QL * hq
                            xs = hf.tile([128, QL], F32, tag="xs", bufs=6)
                            nc.scalar.dma_start(xs[0:64, :],
                                                xin[iA, :, off:off + QL])
                            nc.scalar.dma_start(xs[64:128, :],
                                                xin[iA + 1, :, off:off + QL])
                            xsc = hf.tile([128, QL], F32, tag="xs", bufs=6)
                            nc.scalar.activation(xsc[:], xs[:], AF.Copy,
                                                 scale=rsc2[:])
                            r = hf.tile([128, QL], F32, tag="xs", bufs=6)
                            nc.gpsimd.tensor_tensor(
                                r[:], xsc[:], y2s[p][:, off:off + QL], AO.add)
                            if t == 1:
                                q2v = r[:]
                            else:
                                q2 = hf.tile([128, QL], F32, tag="q2", bufs=4)
                                nc.vector.tensor_tensor(q2[:], r[:],
                                                        Pprev2[hq][:], AO.add)
                                q2v = q2[:]
                            ot = hf.tile([128, QL], F32, tag="ot", bufs=2)
                            nc.vector.tensor_scalar(ot[:], q2v, th2[:],
                                                    None, AO.is_ge)
                            nc.sync.dma_start(outp[iA, :, off:off + QL],
                                              ot[0:64, :])
                            nc.sync.dma_start(outp[iA + 1, :, off:off + QL],
                                              ot[64:128, :])
                            if t < 4:
                                sb2 = hf.tile([128, QL], F16, tag="sb", bufs=2)
                                nc.vector.tensor_scalar(sb2[:], q2v, th2[:],
                                                        None, AO.is_lt)
                                wv2 = hf.tile([128, QL], F32, tag="q2", bufs=4)
                                nc.scalar.activation(wv2[:], q2v, AF.Identity,
                                                     bias=gmw2[:],
                                                     scale=1.0 - alpha2)
                                Pn = hf.tile([128, QL], F32, tag="pp", bufs=6)
                                nc.vector.tensor_tensor(Pn[:], wv2[:],
                                                        sb2[:], AO.mult)
                                Pprev2[hq] = Pn

    nc.compile()
    return nc, names


def _sigmoid(x):
    return 1.0 / (1.0 + np.exp(-float(x)))


def prepare(x, conv1_w, bn1_gamma, bn1_beta, lif1_w, conv2_w, bn2_gamma,
            bn2_beta, lif2_w):
    x = np.ascontiguousarray(np.asarray(x, np.float32))
    conv1_w = np.asarray(conv1_w, np.float32)
    conv2_w = np.asarray(conv2_w, np.float32)

    a1 = _sigmoid(np.asarray(lif1_w).reshape(-1)[0])
    a2 = _sigmoid(np.asarray(lif2_w).reshape(-1)[0])

    key = (round(a1, 12), round(a2, 12))
    if key not in _prog_cache:
        _prog_cache[key] = _build(a1, a2)
    nc, names = _prog_cache[key]

    # fp16 hi/lo split of x, padded planes (encoding only; exact split)
    xh = x.astype(np.float16)
    xl = (x - xh.astype(np.float32)).astype(np.float16)
    xpad = np.zeros((T, B, C, 2, HP, HP), np.float16)
    xpad[:, :, :, 0, 1:57, 1:57] = xh
    xpad[:, :, :, 1, 1:57, 1:57] = xl
    xpad = np.ascontiguousarray(xpad.transpose(0, 1, 3, 2, 4, 5))  # t,b,2,c,hp,hp

    w1h = conv1_w.astype(np.float16)
    w1l = (conv1_w - w1h.astype(np.float32)).astype(np.float16)
    w2h = conv2_w.astype(np.float16)
    w2l = (conv2_w - w2h.astype(np.float32)).astype(np.float16)

    def tapstack(wtop, wbot):
        out = np.zeros((128, 9 * 64), np.float16)
        for a in range(9):
            di, dj = a // 3, a % 3
            out[0:64, a * 64:(a + 1) * 64] = wtop[:, :, di, dj].T
            out[64:128, a * 64:(a + 1) * 64] = wbot[:, :, di, dj].T
        return out

    w1a_np = tapstack(w1h, w1h)
    w1b_np = tapstack(w1l, w1l)
    w2a_np = tapstack(w2h, w2l)

    def dup(v):
        v = np.asarray(v, np.float32).reshape(64)
        return np.concatenate([v, v])

    cpar_np = np.zeros((128, 8), np.float32)
    cpar_np[:, 0] = dup(bn1_gamma)
    cpar_np[:, 1] = dup(bn1_beta)
    cpar_np[:, 2] = dup(bn2_gamma)
    cpar_np[:, 3] = dup(bn2_beta)
    cpar_np[:, 4] = 1.0 / (a1 * dup(bn1_gamma))
    cpar_np[:, 5] = 1.0 / (a2 * dup(bn2_gamma))
    cpar_np[:, 6] = 1.0 / dup(bn1_gamma)
    cpar_np[:, 7] = 1.0 / dup(bn2_gamma)

    in_maps = []
    for k in range(NCORES):
        xta_np = np.ascontiguousarray(
            xpad[:, 4 * k:4 * k + 4].reshape(NIMG, 2, 64, PP))
        xin_np = np.ascontiguousarray(
            x[:, 4 * k:4 * k + 4].reshape(NIMG, 64, PIX))
        in_maps.append({
            names['xta']: xta_np,
            names['xin']: xin_np,
            names['w1a']: w1a_np,
            names['w1b']: w1b_np,
            names['w2a']: w2a_np,
            names['cpar']: cpar_np,
        })

    return nc, names, in_maps


def kernel(**inputs):
    from concourse.bass_utils import run_bass_kernel_spmd
    nc, names, in_maps = prepare(**inputs)
    res = run_bass_kernel_spmd(nc, in_maps, core_ids=list(range(NCORES)))
    global LAST_RES, LAST_NAMES
    LAST_RES, LAST_NAMES = res, names
    out = np.empty((T, B, C, H, W), np.float32)
    for k in range(NCORES):
        o = res.results[k][names['outp']]
        out[:, 4 * k:4 * k + 4] = o.reshape(T, BL, C, H, W)
    return out


if __name__ == "__main__":
    rng = np.random.default_rng(0)
    xs = rng.standard_normal((T, B, C, H, W)).astype(np.float32)
    w1 = (rng.standard_normal((64, 64, 3, 3)) * 0.05).astype(np.float32)
    w2 = (rng.standard_normal((64, 64, 3, 3)) * 0.05).astype(np.float32)
    o = kernel(xs, w1, np.ones(64, np.float32), np.zeros(64, np.float32),
               np.zeros(1, np.float32), w2, np.ones(64, np.float32),
               np.zeros(64, np.float32), np.zeros(1, np.float32))
    print("ran:", o.shape, float(o.mean()))



# revision 24
# speedup vs baseline: 1.4564x; 1.4564x over previous
"""Trainium2 Bass kernel for nn_BasicBlock (spiking CNN block).

Sharding: data-parallel over batch B across 8 NeuronCores (4 batch x 4
timesteps = 16 images per core); BN batch stats via tiny AllReduce.

Per core (vs the 27-pass baseline this runs 16 pass-equivalents):
- conv1 main (W1hi . xhi, fp16): taps row-paired via planes laid out as
  [xhi ; xhi shifted one row] so K=128 contracts two taps -> 6 passes.
- conv1 corr (W1hi . xlo + W1lo . xhi): fp8e4 DoubleRow matmuls (0.5
  cyc/row) over planes [e4m3(x) ; e4m3(xlo*2^11)], two taps per
  instruction, M=128 with half-zero weight columns so each image's
  correction lands on its own partition half of ONE corr PSUM bank.
  Streams flat at the padded pitch (58); pad columns hold garbage that
  the evacuation never reads. ~2.5 pass-equivalents.
- conv2: same structure on spike planes [s1 ; s1 shifted] (s1 exact in
  fp16 AND fp8): hi 6 passes + W2lo fp8-DR corr 1.5 pass-equivalents.
- Evacuation: Act copy (main psum) -> strip, DVE stt strip += corr*2^-11
  (accum_out -> BN sums), Act square (accum_out -> BN sumsq).
- PLIF scans in "q-space" with negated-state fusion: s and the carried
  state are scalar_tensor_tensor ops; reset mask via Act (s-1).
- Residual+LIF2 spread across Act (x*rsc2, mask), Pool (adds), DVE
  (compares + state); spikes written out as fp16 and cast on host.
"""
import sys
sys.path.insert(0, '/opt/trn_rl_repo')

import numpy as np

T, B, C, H, W = 4, 32, 64, 56, 56
NCORES = 8
BL = B // NCORES            # 4 local batch samples
NIMG = T * BL               # 16 images per core
HP = W + 2                  # 58
PPP = HP * HP + 4           # padded plane + flat-stream overrun guard (3368)
PIX = H * W                 # 3136
NCH = 7                     # conv chunks per image (8 rows each)
CHW = 8 * W                 # 448 compact chunk
CFL = 8 * HP                # 464 flat chunk span
NPAIR = 8                   # image pairs per core
EPS = 1e-5
NG = float((T * B) * PIX)   # 401408
QL = 14 * W                 # LIF quarter-strip length (784)
NQ = 4
CSC = 2048.0                # 2^11 fp8 correction scale

# (di, dj) slice per fp16 main set: di=0 -> taps (0,dj)+(1,dj) paired via
# the shifted upper half; di=2 -> tap (2,dj) solo (upper weights zero).
MAIN_SETS = [(0, 0), (0, 1), (0, 2), (2, 0), (2, 1), (2, 2)]
# conv1 fp8-DR sets: (di, dj, dk) -> k-tile1 at (di,dj), k-tile2 at +dk
# covering tap pairs ((0,j),(1,j))x3, ((2,0),(2,2)) [stride 2; odd k-tile
# strides crash the ifmap fetcher], ((2,1), zero)
DR1_SETS = [(0, 0, HP), (0, 1, HP), (0, 2, HP), (2, 0, 2), (2, 1, 2)]
# conv2 fp8-DR sets: k-tiles 2 rows apart cover taps (0,j),(1,j),(2,j),x0
DR2_SETS = [(0, 0, 2 * HP), (0, 1, 2 * HP), (0, 2, 2 * HP)]

_prog_cache = {}
DBG = False
NO_CC = False
PHASES = 3
TRACE = False
LAST_RES = None
LAST_NAMES = None
LAST_EXEC_NS = None


def _build(alpha1, alpha2):
    import concourse.mybir as mybir
    import concourse.tile as tile
    from concourse.ap import AP
    from concourse import bacc

    F32 = mybir.dt.float32
    F16 = mybir.dt.float16
    F8 = mybir.dt.float8e4
    AO = mybir.AluOpType
    AF = mybir.ActivationFunctionType
    AX = mybir.AxisListType
    DRM = mybir.MatmulPerfMode.DoubleRow

    c1 = 1.0 - alpha1
    c2 = 1.0 - alpha2

    def sub_ap(base, extra_off, dims):
        b = base.copy()
        return AP(b.tensor, b.offset + extra_off,
                  [list(b.ap[0])] + [list(d) for d in dims])

    nc = bacc.Bacc(None, target_bir_lowering=False)
    names = {}

    with tile.TileContext(nc) as tc:
        with tc.tile_pool(name="dram", bufs=1, space="DRAM") as dram:
            pa = dram.tile([NIMG, 128, PPP], F16, kind="ExternalInput")
            pb = dram.tile([NIMG, 128, PPP], F8, kind="ExternalInput")
            xin = dram.tile([NIMG, 64, PIX], F32, kind="ExternalInput")
            w1m = dram.tile([128, 6, 64], F16, kind="ExternalInput")
            w1c = dram.tile([128, 5, 2, 192], F8, kind="ExternalInput")
            w2a = dram.tile([128, 9 * 64], F16, kind="ExternalInput")
            cpar = dram.tile([128, 8], F32, kind="ExternalInput")
            outp = dram.tile([NIMG, 64, PIX], F16, kind="ExternalOutput")
            names.update(pa=pa.name, pb=pb.name, xin=xin.name, w1m=w1m.name,
                         w1c=w1c.name, w2a=w2a.name,
                         cpar=cpar.name, outp=outp.name)
            if DBG:
                y1d = dram.tile([NPAIR, 128, PIX], F32, kind="ExternalOutput")
                y2d = dram.tile([NPAIR, 128, PIX], F32, kind="ExternalOutput")
                s1d = dram.tile([NPAIR, 128, PIX], F32, kind="ExternalOutput")
                vecd = dram.tile([128, 8], F32, kind="ExternalOutput")
                names.update(y1d=y1d.name, y2d=y2d.name, s1d=s1d.name,
                             vecd=vecd.name)

            with tc.tile_pool(name="dramw", bufs=1, space="DRAM") as dramw, \
                 tc.tile_pool(name="wsb", bufs=1) as wsb, \
                 tc.tile_pool(name="ys", bufs=8) as yspool, \
                 tc.tile_pool(name="plane", bufs=3) as plpool, \
                 tc.tile_pool(name="hfp", bufs=2) as hf, \
                 tc.tile_pool(name="tiny", bufs=42) as tiny, \
                 tc.tile_pool(name="ps", bufs=7, space="PSUM") as ps:

                # ---- static parameter loads
                w1ms = wsb.tile([128, 6, 64], F16, tag="w1m")
                nc.sync.dma_start(w1ms[:], w1m[:])
                w1cs = wsb.tile([128, 5, 2, 192], F8, tag="w1c")
                nc.sync.dma_start(w1cs[:], w1c[:])
                w2as = wsb.tile([128, 9 * 64], F16, tag="w2a")
                nc.sync.dma_start(w2as[:], w2a[:])
                cpars = wsb.tile([128, 8], F32, tag="cpar")
                nc.sync.dma_start(cpars[:], cpar[:])
                negone = wsb.tile([128, 1], F32, tag="negone")
                nc.vector.memset(negone[:], -1.0)
                zq = wsb.tile([128, QL], F32, tag="zq")
                nc.vector.memset(zq[:], 0.0)
                sums1 = wsb.tile([128, 56], F32, tag="sums1")
                sums1q = wsb.tile([128, 56], F32, tag="sums1q")
                sums2 = wsb.tile([128, 56], F32, tag="sums2")
                sums2q = wsb.tile([128, 56], F32, tag="sums2q")
                if PHASES < 2:
                    nc.vector.memset(sums2[:], 0.0)
                    nc.vector.memset(sums2q[:], 0.0)

                def conv_img_pair(plA, plB, plA8, plB8, wm, wc, dr_sets,
                                  dst_strip, sums_t, sumsq_t, pcol):
                    """One image pair: per chunk, 6 fp16 tap-paired matmuls
                    per image into a main psum + fp8 DoubleRow correction
                    instructions (both images) into one flat corr psum, then
                    the 3-op evacuation."""
                    plAr = plA[:, 0:HP * HP].rearrange("p (r w) -> p r w", w=HP)
                    plBr = plB[:, 0:HP * HP].rearrange("p (r w) -> p r w", w=HP)
                    ndr = len(dr_sets)
                    for cth in range(NCH):
                        r0 = 8 * cth
                        pm = ps.tile([128, CHW], F32, tag="psm", bufs=4,
                                     name=f"psm{cth & 1}")
                        pc = ps.tile([128, CFL], F32, tag="psc", bufs=3,
                                     name=f"psc{cth & 1}")
                        for si, (di, dj) in enumerate(MAIN_SETS):
                            for j, plr in enumerate((plAr, plBr)):
                                rhs = plr[:, r0 + di:r0 + di + 8, dj:dj + W]
                                out = pm[64 * j:64 * (j + 1), :] \
                                    .rearrange("p (r w) -> p r w", r=8)
                                nc.tensor.matmul(
                                    out, wm[:, si, :], rhs,
                                    start=(si == 0), stop=(si == 5),
                                    tile_position=(0, 64 * j),
                                    skip_group_check=True)
                        idx = 0
                        for j, pl8 in enumerate((plA8, plB8)):
                            co = 64 * (1 - j)
                            for i, (di, dj, dk) in enumerate(dr_sets):
                                base = (r0 + di) * HP + dj
                                rhs = sub_ap(pl8[:], base, [(dk, 2), (1, CFL)])
                                nc.tensor.matmul(
                                    pc[:], wc[:, i, :, co:co + 128], rhs,
                                    start=(idx == 0), stop=(idx == 2 * ndr - 1),
                                    perf_mode=DRM, tile_position=(0, 0),
                                    skip_group_check=True)
                                idx += 1
                        # evacuation: copy main, add scaled corr, square
                        sl = dst_strip[:, CHW * cth:CHW * (cth + 1)]
                        slv = sl.rearrange("p (r w) -> p r w", w=W)
                        nc.scalar.activation(sl, pm[:], AF.Copy)
                        pcv = sub_ap(pc[:], 0, [(HP, 8), (1, W)])
                        nc.vector.scalar_tensor_tensor(
                            slv, pcv, 1.0 / CSC, slv, AO.mult, AO.add,
                            accum_out=sums_t[:, pcol * 7 + cth:pcol * 7 + cth + 1])
                        jk = hf.tile([128, CHW], F16, tag="jk", bufs=2)
                        nc.scalar.activation(
                            jk[:], sl, AF.Square,
                            accum_out=sumsq_t[:, pcol * 7 + cth:pcol * 7 + cth + 1])


                def conv_img_pair2(plA, plB, dst_strip, sums_t, sumsq_t, pcol):
                    """Baseline conv2: [W2hi;W2lo] x [s1;s1], 9 taps, K=128."""
                    plAr = plA[:, 0:HP * HP].rearrange("p (r w) -> p r w", w=HP)
                    plBr = plB[:, 0:HP * HP].rearrange("p (r w) -> p r w", w=HP)
                    for cth in range(NCH):
                        r0 = 8 * cth
                        pm = ps.tile([128, CHW], F32, tag="psm", bufs=4,
                                     name=f"p2m{cth & 1}")
                        for a in range(9):
                            di, dj = a // 3, a % 3
                            for j, plr in enumerate((plAr, plBr)):
                                rhs = plr[:, r0 + di:r0 + di + 8, dj:dj + W]
                                out = pm[64 * j:64 * (j + 1), :] \
                                    .rearrange("p (r w) -> p r w", r=8)
                                nc.tensor.matmul(
                                    out, w2as[:, 64 * a:64 * (a + 1)], rhs,
                                    start=(a == 0), stop=(a == 8),
                                    tile_position=(0, 64 * j),
                                    skip_group_check=True)
                        sl = dst_strip[:, CHW * cth:CHW * (cth + 1)]
                        nc.scalar.activation(
                            sl, pm[:], AF.Copy,
                            accum_out=sums_t[:, pcol * 7 + cth:pcol * 7 + cth + 1])
                        jk = hf.tile([128, CHW], F16, tag="jk", bufs=2)
                        nc.vector.scalar_tensor_tensor(
                            jk[:], sl, 1.0, sl, AO.bypass, AO.mult,
                            accum_out=sumsq_t[:, pcol * 7 + cth:pcol * 7 + cth + 1])

                # ================= phase A: conv1 =================
                y1s = []
                for p in range(NPAIR):
                    tt_, bp = p // 2, p % 2
                    iA = tt_ * 4 + bp * 2
                    pls, pl8s_ = [], []
                    for j in range(2):
                        i = iA + j
                        ta = plpool.tile([128, PPP], F16, tag="plf16")
                        nc.sync.dma_start(ta[:], pa[i])
                        t8 = plpool.tile([128, PPP], F8, tag="plf8")
                        nc.sync.dma_start(t8[:], pb[i])
                        pls.append(ta)
                        pl8s_.append(t8)
                    strip = yspool.tile([128, PIX], F32, tag="ys")
                    y1s.append(strip)
                    conv_img_pair(pls[0], pls[1], pl8s_[0], pl8s_[1],
                                  w1ms, w1cs, DR1_SETS, strip, sums1, sums1q, p)
                    if DBG:
                        nc.sync.dma_start(y1d[p], strip[:])

                # ---- stats1 allreduce
                cc1i = dramw.tile([128, 2], F32)
                cc1o = dramw.tile([128, 2], F32, addr_space="Shared")
                acc1 = tiny.tile([128, 2], F32, tag="acc")
                nc.vector.tensor_reduce(acc1[:, 0:1], sums1[:], AX.X, AO.add)
                nc.vector.tensor_reduce(acc1[:, 1:2], sums1q[:], AX.X, AO.add)
                nc.sync.dma_start(cc1i[:], acc1[:])
                if NO_CC:
                    nc.sync.dma_start(cc1o[:], cc1i[:])
                else:
                    nc.gpsimd.collective_compute(
                        "AllReduce", AO.add, ins=[cc1i[:]], outs=[cc1o[:]],
                        replica_groups=[list(range(NCORES))])
                g1 = tiny.tile([128, 2], F32, tag="acc")
                nc.sync.dma_start(g1[:], cc1o[:])

                def stats_block(g, gamma, beta, rga, rgam, alpha):
                    gr = tiny.tile([128, 2], F32, tag="acc")
                    nc.sync.dma_start(gr[0:64, :], g[64:128, :])
                    nc.sync.dma_start(gr[64:128, :], g[0:64, :])
                    tot = tiny.tile([128, 2], F32, tag="acc")
                    nc.vector.tensor_tensor(tot[:], g[:], gr[:], AO.add)
                    mean = tiny.tile([128, 1], F32, tag="t1")
                    nc.vector.tensor_scalar(mean[:], tot[:, 0:1], 1.0 / NG,
                                            None, AO.mult)
                    msq = tiny.tile([128, 1], F32, tag="t1")
                    nc.vector.tensor_scalar(msq[:], tot[:, 1:2], 1.0 / NG,
                                            None, AO.mult)
                    m2 = tiny.tile([128, 1], F32, tag="t1")
                    nc.vector.scalar_tensor_tensor(m2[:], mean[:], 1.0, mean[:],
                                                   AO.bypass, AO.mult)
                    var = tiny.tile([128, 1], F32, tag="t1")
                    nc.vector.tensor_tensor(var[:], msq[:], m2[:], AO.subtract)
                    epst = tiny.tile([128, 1], F32, tag="t1")
                    nc.vector.memset(epst[:], EPS)
                    std = tiny.tile([128, 1], F32, tag="t1")
                    nc.scalar.activation(std[:], var[:], AF.Sqrt, bias=epst[:])
                    rstd = tiny.tile([128, 1], F32, tag="t1")
                    nc.vector.reciprocal(rstd[:], std[:])
                    sc = tiny.tile([128, 1], F32, tag="t1")
                    nc.vector.tensor_tensor(sc[:], gamma, rstd[:], AO.mult)
                    nmsc = tiny.tile([128, 1], F32, tag="t1")
                    nc.vector.scalar_tensor_tensor(nmsc[:], mean[:], -1.0, sc[:],
                                                   AO.mult, AO.mult)
                    bi = tiny.tile([128, 1], F32, tag="t1")
                    nc.vector.tensor_tensor(bi[:], beta, nmsc[:], AO.add)
                    stdrg = tiny.tile([128, 1], F32, tag="t1")
                    nc.vector.tensor_tensor(stdrg[:], std[:], rga, AO.mult)
                    nbst = tiny.tile([128, 1], F32, tag="t1")
                    nc.vector.scalar_tensor_tensor(nbst[:], bi[:], -alpha,
                                                   stdrg[:], AO.mult, AO.mult)
                    th = tiny.tile([128, 1], F32, tag="t1")
                    nc.vector.tensor_tensor(th[:], stdrg[:], nbst[:], AO.add)
                    bstd = tiny.tile([128, 1], F32, tag="t1")
                    nc.vector.tensor_tensor(bstd[:], bi[:], std[:], AO.mult)
                    gamv = tiny.tile([128, 1], F32, tag="t1")
                    nc.vector.tensor_tensor(gamv[:], bstd[:], rgam, AO.mult)
                    rscv = tiny.tile([128, 1], F32, tag="t1")
                    nc.vector.tensor_tensor(rscv[:], std[:], rgam, AO.mult)
                    return th, gamv, rscv

                th1, gm1, _rsc1 = stats_block(
                    g1, cpars[:, 0:1], cpars[:, 1:2], cpars[:, 4:5],
                    cpars[:, 6:7], alpha1)
                # wv1 bias: c1*gm1 so wv = c1*(q + gm1); Pneg = wv*(s-1) = -P
                gm1c1 = tiny.tile([128, 1], F32, tag="t1")
                nc.vector.tensor_scalar(gm1c1[:], gm1[:], c1, None, AO.mult)
                if DBG:
                    nc.sync.dma_start(vecd[:, 0:1], th1[:])
                    nc.sync.dma_start(vecd[:, 1:2], gm1[:])
                    nc.sync.dma_start(vecd[:, 4:5], acc1[:, 0:1])
                    nc.sync.dma_start(vecd[:, 5:6], acc1[:, 1:2])

                # ============ phase B + C: LIF1 + conv2 ============
                y2s = [None] * NPAIR
                for bp in range(2 if PHASES >= 2 else 0):
                    Pneg = [None] * NQ
                    for t in range(1, 5):
                        p = (t - 1) * 2 + bp
                        s1tq = []
                        for hq in range(NQ):
                            off = QL * hq
                            ysl = y1s[p][:, off:off + QL]
                            s1t = hf.tile([128, QL], F16, tag="s1t", bufs=3)
                            if t == 1:
                                nc.vector.tensor_scalar(s1t[:], ysl, th1[:],
                                                        None, AO.is_ge)
                            else:
                                # s = (y1 - th) >= Pneg  <=>  y1 + P >= th
                                nc.vector.scalar_tensor_tensor(
                                    s1t[:], ysl, th1[:], Pneg[hq][:],
                                    AO.subtract, AO.is_ge)
                            s1tq.append(s1t)
                            if DBG:
                                nc.gpsimd.dma_start(
                                    s1d[p, :, off:off + QL], s1t[:])
                            if t < 4:
                                mng = hf.tile([128, QL], F16, tag="mng", bufs=4)
                                nc.scalar.activation(mng[:], s1t[:],
                                                     AF.Identity,
                                                     bias=negone[:])
                                # v = q + gm1 = (y1 + gm1) - Pneg
                                v = hf.tile([128, QL], F32, tag="wvn", bufs=3)
                                if t == 1:
                                    nc.vector.tensor_scalar(
                                        v[:], ysl, gm1[:], None, AO.add)
                                else:
                                    nc.vector.scalar_tensor_tensor(
                                        v[:], ysl, gm1[:], Pneg[hq][:],
                                        AO.add, AO.subtract)
                                Pn = hf.tile([128, QL], F32, tag="pp", bufs=9)
                                # (c1*v) * (s-1) = -c1*v*(1-s) = -P'
                                nc.vector.scalar_tensor_tensor(
                                    Pn[:], v[:], c1, mng[:], AO.mult, AO.mult)
                                Pneg[hq] = Pn
                        # assemble spike planes [s1 ; s1] (duplicated)
                        sa_pair = []
                        for j in range(2):
                            sa = plpool.tile([128, PPP], F16, tag="plf16")
                            sar = sa[:, 0:HP * HP].rearrange(
                                "p (r w) -> p r w", w=HP)
                            for hq in range(NQ):
                                s1t = s1tq[hq]
                                src = s1t[64 * j:64 * (j + 1), :] \
                                    .rearrange("p (r w) -> p r w", w=W)
                                rl = 1 + 14 * hq
                                nc.sync.dma_start(
                                    sar[0:64, rl:rl + 14, 1:1 + W], src)
                                nc.sync.dma_start(
                                    sar[64:128, rl:rl + 14, 1:1 + W], src)
                            sa_pair.append(sa)
                        strip2 = yspool.tile([128, PIX], F32, tag="ys")
                        y2s[p] = strip2
                        conv_img_pair2(sa_pair[0], sa_pair[1],
                                       strip2, sums2, sums2q, p)
                        if DBG:
                            nc.sync.dma_start(y2d[p], strip2[:])

                # ---- stats2 allreduce
                cc2i = dramw.tile([128, 2], F32)
                cc2o = dramw.tile([128, 2], F32, addr_space="Shared")
                acc2 = tiny.tile([128, 2], F32, tag="acc")
                nc.vector.tensor_reduce(acc2[:, 0:1], sums2[:], AX.X, AO.add)
                nc.vector.tensor_reduce(acc2[:, 1:2], sums2q[:], AX.X, AO.add)
                nc.sync.dma_start(cc2i[:], acc2[:])
                if NO_CC:
                    nc.sync.dma_start(cc2o[:], cc2i[:])
                else:
                    nc.gpsimd.collective_compute(
                        "AllReduce", AO.add, ins=[cc2i[:]], outs=[cc2o[:]],
                        replica_groups=[list(range(NCORES))])
                g2 = tiny.tile([128, 2], F32, tag="acc")
                nc.sync.dma_start(g2[:], cc2o[:])
                th2, gm2, rsc2 = stats_block(
                    g2, cpars[:, 2:3], cpars[:, 3:4], cpars[:, 5:6],
                    cpars[:, 7:8], alpha2)
                if DBG:
                    nc.sync.dma_start(vecd[:, 2:3], th2[:])
                    nc.sync.dma_start(vecd[:, 3:4], gm2[:])

                # ============ phase D: residual + LIF2 ============
                # t-outer emission: 8 independent (bp,hq) chains per step
                Pneg2 = {}
                for t in range(1 if PHASES >= 3 else 5, 5):
                    for bp in range(2):
                        p = (t - 1) * 2 + bp
                        iA = (t - 1) * 4 + bp * 2
                        for hq in range(NQ):
                            off = QL * hq
                            # wneg = Pneg - y2 on Pool, off the x path
                            wneg = hf.tile([128, QL], F32, tag="wvn", bufs=3)
                            if t == 1:
                                nc.gpsimd.tensor_tensor(
                                    wneg[:], zq[:], y2s[p][:, off:off + QL],
                                    AO.subtract)
                            else:
                                nc.gpsimd.tensor_tensor(
                                    wneg[:], Pneg2[bp, hq][:],
                                    y2s[p][:, off:off + QL], AO.subtract)
                            xs = hf.tile([128, QL], F32, tag="xs", bufs=4)
                            nc.sync.dma_start(
                                xs[:],
                                xin[iA:iA + 2, :, off:off + QL]
                                .rearrange("a p q -> (a p) q"))
                            # xsc = x*rsc2 in-place (Act)
                            nc.scalar.activation(xs[:], xs[:], AF.Copy,
                                                 scale=rsc2[:])
                            ot = hf.tile([128, QL], F16, tag="ot", bufs=4)
                            # s: (xsc - th) >= wneg  <=>  xsc + y2 + P >= th
                            nc.vector.scalar_tensor_tensor(
                                ot[:], xs[:], th2[:], wneg[:],
                                AO.subtract, AO.is_ge)
                            nc.scalar.dma_start(
                                outp[iA:iA + 2, :, off:off + QL]
                                .rearrange("a p q -> (a p) q"), ot[:])
                            if t < 4:
                                mng = hf.tile([128, QL], F16, tag="mng",
                                              bufs=4)
                                nc.scalar.activation(mng[:], ot[:],
                                                     AF.Identity,
                                                     bias=negone[:])
                                # u = q + gm2 = (xsc + gm2) - wneg, in-place
                                nc.vector.scalar_tensor_tensor(
                                    xs[:], xs[:], gm2[:], wneg[:],
                                    AO.add, AO.subtract)
                                Pn = hf.tile([128, QL], F32, tag="pp", bufs=9)
                                # (c2*u) * (s-1) = -c2*u*(1-s) = -P'
                                nc.vector.scalar_tensor_tensor(
                                    Pn[:], xs[:], c2, mng[:], AO.mult, AO.mult)
                                Pneg2[bp, hq] = Pn

    nc.compile()
    return nc, names


def _sigmoid(x):
    return 1.0 / (1.0 + np.exp(-float(x)))


def prepare(x, conv1_w, bn1_gamma, bn1_beta, lif1_w, conv2_w, bn2_gamma,
            bn2_beta, lif2_w):
    import ml_dtypes
    E4 = ml_dtypes.float8_e4m3

    x = np.ascontiguousarray(np.asarray(x, np.float32))
    conv1_w = np.asarray(conv1_w, np.float32)
    conv2_w = np.asarray(conv2_w, np.float32)

    a1 = _sigmoid(np.asarray(lif1_w).reshape(-1)[0])
    a2 = _sigmoid(np.asarray(lif2_w).reshape(-1)[0])

    key = (round(a1, 12), round(a2, 12))
    if key not in _prog_cache:
        _prog_cache[key] = _build(a1, a2)
    nc, names = _prog_cache[key]

    xh = x.astype(np.float16)
    xl = x - xh.astype(np.float32)

    # fp16 main planes: [xhi padded ; xhi shifted up one row]
    pa_np = np.zeros((T, B, 128, HP, HP), np.float16)
    pa_np[:, :, 0:64, 1:57, 1:57] = xh
    pa_np[:, :, 64:128, 0:56, 1:57] = xh
    # fp8 corr planes: [e4m3(x) ; e4m3(xl*2^11)], same (unshifted) layout
    pb_np = np.zeros((T, B, 128, HP, HP), E4)
    pb_np[:, :, 0:64, 1:57, 1:57] = x.astype(E4)
    pb_np[:, :, 64:128, 1:57, 1:57] = (xl * CSC).astype(E4)

    def pad_flat(arr):
        # [T,B,128,HP,HP] -> [T,B,128,PPP] with zero tail
        t_, b_, p_, _, _ = arr.shape
        out = np.zeros((t_, b_, p_, PPP), arr.dtype)
        out[..., :HP * HP] = arr.reshape(t_, b_, p_, HP * HP)
        return out

    pa_np = pad_flat(pa_np)
    pb_np = pad_flat(pb_np)

    w1h = conv1_w.astype(np.float16)
    w1l = conv1_w - w1h.astype(np.float32)
    w2h = conv2_w.astype(np.float16)
    w2l = conv2_w - w2h.astype(np.float32)

    def main_stack(wh):
        out = np.zeros((128, 6, 64), np.float16)
        for s, (di, dj) in enumerate(MAIN_SETS):
            if di == 0:
                out[0:64, s] = wh[:, :, 0, dj].T
                out[64:128, s] = wh[:, :, 1, dj].T
            else:
                out[0:64, s] = wh[:, :, 2, dj].T
        return out

    w1m_np = main_stack(w1h)

    def tapstack(wtop, wbot):
        out = np.zeros((128, 9 * 64), np.float16)
        for a in range(9):
            di, dj = a // 3, a % 3
            out[0:64, a * 64:(a + 1) * 64] = wtop[:, :, di, dj].T
            out[64:128, a * 64:(a + 1) * 64] = wbot[:, :, di, dj].T
        return out

    w2a_np = tapstack(w2h, w2l.astype(np.float16))

    # conv1 DR weights: k-tile block rows 0:64 multiply e4m3(x) -> W1l_s,
    # rows 64:128 multiply e4m3(xl*2^11) -> W1h8; active cols at 64:128.
    w1l_s = (w1l * CSC).astype(E4)
    w1h8 = conv1_w.astype(E4)
    w1c_np = np.zeros((128, 5, 2, 192), E4)
    dr1_taps = [((0, 0), (1, 0)), ((0, 1), (1, 1)), ((0, 2), (1, 2)),
                ((2, 0), (2, 2)), ((2, 1), None)]
    for i, (ta, tb) in enumerate(dr1_taps):
        for kk, tap in enumerate((ta, tb)):
            if tap is None:
                continue
            di, dj = tap
            w1c_np[0:64, i, kk, 64:128] = w1l_s[:, :, di, dj].astype(
                np.float32).T.astype(E4)
            w1c_np[64:128, i, kk, 64:128] = w1h8[:, :, di, dj].astype(
                np.float32).T.astype(E4)

    def dup(v):
        v = np.asarray(v, np.float32).reshape(64)
        return np.concatenate([v, v])

    cpar_np = np.zeros((128, 8), np.float32)
    cpar_np[:, 0] = dup(bn1_gamma)
    cpar_np[:, 1] = dup(bn1_beta)
    cpar_np[:, 2] = dup(bn2_gamma)
    cpar_np[:, 3] = dup(bn2_beta)
    cpar_np[:, 4] = 1.0 / (a1 * dup(bn1_gamma))
    cpar_np[:, 5] = 1.0 / (a2 * dup(bn2_gamma))
    cpar_np[:, 6] = 1.0 / dup(bn1_gamma)
    cpar_np[:, 7] = 1.0 / dup(bn2_gamma)

    in_maps = []
    for k in range(NCORES):
        sl = slice(4 * k, 4 * k + 4)
        pa_k = np.ascontiguousarray(pa_np[:, sl].reshape(NIMG, 128, PPP))
        pb_k = np.ascontiguousarray(pb_np[:, sl].reshape(NIMG, 128, PPP))
        xin_k = np.ascontiguousarray(x[:, sl].reshape(NIMG, 64, PIX))
        in_maps.append({
            names['pa']: pa_k,
            names['pb']: pb_k.view(np.uint8),
            names['xin']: xin_k,
            names['w1m']: w1m_np,
            names['w1c']: w1c_np.view(np.uint8),
            names['w2a']: w2a_np,
            names['cpar']: cpar_np,
        })

    return nc, names, in_maps


def kernel(**inputs):
    from concourse.bass_utils import run_bass_kernel_spmd
    nc, names, in_maps = prepare(**inputs)
    res = run_bass_kernel_spmd(nc, in_maps, core_ids=list(range(NCORES)))
    global LAST_RES, LAST_NAMES
    LAST_RES, LAST_NAMES = res, names
    out = np.empty((T, B, C, H, W), np.float32)
    for k in range(NCORES):
        o = np.asarray(res.results[k][names['outp']], np.float32)
        out[:, 4 * k:4 * k + 4] = o.reshape(T, BL, C, H, W)
    return out


if __name__ == "__main__":
    rng = np.random.default_rng(0)
    xs = rng.standard_normal((T, B, C, H, W)).astype(np.float32)
    w1 = (rng.standard_normal((64, 64, 3, 3)) * 0.05).astype(np.float32)
    w2 = (rng.standard_normal((64, 64, 3, 3)) * 0.05).astype(np.float32)
    o = kernel(xs, w1, np.ones(64, np.float32), np.zeros(64, np.float32),
               np.zeros(1, np.float32), w2, np.ones(64, np.float32),
               np.zeros(64, np.float32), np.zeros(1, np.float32))
    print("ran:", o.shape, float(o.mean()))


# revision 27
# speedup vs baseline: 1.5382x; 1.0562x over previous
"""Trainium2 Bass kernel for nn_BasicBlock (spiking CNN block).

Sharding: data-parallel over batch B across 8 NeuronCores (4 batch x 4
timesteps = 16 images per core); BN batch stats via tiny AllReduce.

Per core (vs the 27-pass baseline this runs 16 pass-equivalents):
- conv1 main (W1hi . xhi, fp16): taps row-paired via planes laid out as
  [xhi ; xhi shifted one row] so K=128 contracts two taps -> 6 passes.
- conv1 corr (W1hi . xlo + W1lo . xhi): fp8e4 DoubleRow matmuls (0.5
  cyc/row) over planes [e4m3(x) ; e4m3(xlo*2^11)], two taps per
  instruction, M=128 with half-zero weight columns so each image's
  correction lands on its own partition half of ONE corr PSUM bank.
  Streams flat at the padded pitch (58); pad columns hold garbage that
  the evacuation never reads. ~2.5 pass-equivalents.
- conv2: same structure on spike planes [s1 ; s1 shifted] (s1 exact in
  fp16 AND fp8): hi 6 passes + W2lo fp8-DR corr 1.5 pass-equivalents.
- Evacuation: Act copy (main psum) -> strip, DVE stt strip += corr*2^-11
  (accum_out -> BN sums), Act square (accum_out -> BN sumsq).
- PLIF scans in "q-space" with negated-state fusion: s and the carried
  state are scalar_tensor_tensor ops; reset mask via Act (s-1).
- Residual+LIF2 spread across Act (x*rsc2, mask), Pool (adds), DVE
  (compares + state); spikes written out as fp16 and cast on host.
"""
import sys
sys.path.insert(0, '/opt/trn_rl_repo')

import numpy as np

T, B, C, H, W = 4, 32, 64, 56, 56
NCORES = 8
BL = B // NCORES            # 4 local batch samples
NIMG = T * BL               # 16 images per core
HP = W + 2                  # 58
PPP = HP * HP + 4           # padded plane + flat-stream overrun guard (3368)
PIX = H * W                 # 3136
NCH = 7                     # conv chunks per image (8 rows each)
CHW = 8 * W                 # 448 compact chunk
CFL = 8 * HP                # 464 flat chunk span
NPAIR = 8                   # image pairs per core
EPS = 1e-5
NG = float((T * B) * PIX)   # 401408
QL = 14 * W                 # LIF quarter-strip length (784)
NQ = 4
CSC = 2048.0                # 2^11 fp8 correction scale

# (di, dj) slice per fp16 main set: di=0 -> taps (0,dj)+(1,dj) paired via
# the shifted upper half; di=2 -> tap (2,dj) solo (upper weights zero).
MAIN_SETS = [(0, 0), (0, 1), (0, 2), (2, 0), (2, 1), (2, 2)]
# conv1 fp8-DR sets: (di, dj, dk) -> k-tile1 at (di,dj), k-tile2 at +dk
# covering tap pairs ((0,j),(1,j))x3, ((2,0),(2,2)) [stride 2; odd k-tile
# strides crash the ifmap fetcher], ((2,1), zero)
DR1_SETS = [(0, 0, HP), (0, 1, HP), (0, 2, HP), (2, 0, 2), (2, 1, 2)]
# conv2 fp8-DR sets: k-tiles 2 rows apart cover taps (0,j),(1,j),(2,j),x0
DR2_SETS = [(0, 0, 2 * HP), (0, 1, 2 * HP), (0, 2, 2 * HP)]

_prog_cache = {}
DBG = False
NO_CC = False
PHASES = 3
TRACE = False
LAST_RES = None
LAST_NAMES = None
LAST_EXEC_NS = None


def _build(alpha1, alpha2):
    import concourse.mybir as mybir
    import concourse.tile as tile
    from concourse.ap import AP
    from concourse import bacc

    F32 = mybir.dt.float32
    F16 = mybir.dt.float16
    F8 = mybir.dt.float8e4
    AO = mybir.AluOpType
    AF = mybir.ActivationFunctionType
    AX = mybir.AxisListType
    DRM = mybir.MatmulPerfMode.DoubleRow

    c1 = 1.0 - alpha1
    c2 = 1.0 - alpha2

    def sub_ap(base, extra_off, dims):
        b = base.copy()
        return AP(b.tensor, b.offset + extra_off,
                  [list(b.ap[0])] + [list(d) for d in dims])

    nc = bacc.Bacc(None, target_bir_lowering=False)
    names = {}

    with tile.TileContext(nc) as tc:
        with tc.tile_pool(name="dram", bufs=1, space="DRAM") as dram:
            pa = dram.tile([NIMG, 128, PPP], F16, kind="ExternalInput")
            pb = dram.tile([NIMG, 128, PPP], F8, kind="ExternalInput")
            xin = dram.tile([NIMG, 64, PIX], F32, kind="ExternalInput")
            w1m = dram.tile([128, 6, 64], F16, kind="ExternalInput")
            w1c = dram.tile([128, 5, 2, 192], F8, kind="ExternalInput")
            w2a = dram.tile([128, 9 * 64], F16, kind="ExternalInput")
            cpar = dram.tile([128, 8], F32, kind="ExternalInput")
            outp = dram.tile([NIMG, 64, PIX], F16, kind="ExternalOutput")
            names.update(pa=pa.name, pb=pb.name, xin=xin.name, w1m=w1m.name,
                         w1c=w1c.name, w2a=w2a.name,
                         cpar=cpar.name, outp=outp.name)
            if DBG:
                y1d = dram.tile([NPAIR, 128, PIX], F32, kind="ExternalOutput")
                y2d = dram.tile([NPAIR, 128, PIX], F32, kind="ExternalOutput")
                s1d = dram.tile([NPAIR, 128, PIX], F32, kind="ExternalOutput")
                vecd = dram.tile([128, 8], F32, kind="ExternalOutput")
                names.update(y1d=y1d.name, y2d=y2d.name, s1d=s1d.name,
                             vecd=vecd.name)

            with tc.tile_pool(name="dramw", bufs=1, space="DRAM") as dramw, \
                 tc.tile_pool(name="wsb", bufs=1) as wsb, \
                 tc.tile_pool(name="ys", bufs=8) as yspool, \
                 tc.tile_pool(name="plane", bufs=4) as plpool, \
                 tc.tile_pool(name="hfp", bufs=2) as hf, \
                 tc.tile_pool(name="tiny", bufs=24) as tiny, \
                 tc.tile_pool(name="ps", bufs=7, space="PSUM") as ps:

                # ---- static parameter loads
                w1ms = wsb.tile([128, 6, 64], F16, tag="w1m")
                nc.sync.dma_start(w1ms[:], w1m[:])
                w1cs = wsb.tile([128, 5, 2, 192], F8, tag="w1c")
                nc.sync.dma_start(w1cs[:], w1c[:])
                w2as = wsb.tile([128, 9 * 64], F16, tag="w2a")
                nc.sync.dma_start(w2as[:], w2a[:])
                cpars = wsb.tile([128, 8], F32, tag="cpar")
                nc.sync.dma_start(cpars[:], cpar[:])
                negone = wsb.tile([128, 1], F32, tag="negone")
                nc.vector.memset(negone[:], -1.0)
                zq = wsb.tile([128, QL], F32, tag="zq")
                nc.vector.memset(zq[:], 0.0)
                epst = wsb.tile([128, 1], F32, tag="epst")
                nc.vector.memset(epst[:], EPS)
                sums1 = wsb.tile([128, 56], F32, tag="sums1")
                sums1q = wsb.tile([128, 56], F32, tag="sums1q")
                sums2 = wsb.tile([128, 56], F32, tag="sums2")
                sums2q = wsb.tile([128, 56], F32, tag="sums2q")
                if PHASES < 2:
                    nc.vector.memset(sums2[:], 0.0)
                    nc.vector.memset(sums2q[:], 0.0)

                def conv_img_pair(plA, plB, plA8, plB8, wm, wc, dr_sets,
                                  dst_strip, sums_t, sumsq_t, pcol):
                    """One image pair: per chunk, 6 fp16 tap-paired matmuls
                    per image into a main psum + fp8 DoubleRow correction
                    instructions (both images) into one flat corr psum, then
                    the 3-op evacuation."""
                    plAr = plA[:, 0:HP * HP].rearrange("p (r w) -> p r w", w=HP)
                    plBr = plB[:, 0:HP * HP].rearrange("p (r w) -> p r w", w=HP)
                    ndr = len(dr_sets)
                    for cth in range(NCH):
                        r0 = 8 * cth
                        pm = ps.tile([128, CHW], F32, tag="psm", bufs=4,
                                     name=f"psm{cth & 1}")
                        pc = ps.tile([128, CFL], F32, tag="psc", bufs=3,
                                     name=f"psc{cth & 1}")
                        for si, (di, dj) in enumerate(MAIN_SETS):
                            for j, plr in enumerate((plAr, plBr)):
                                rhs = plr[:, r0 + di:r0 + di + 8, dj:dj + W]
                                out = pm[64 * j:64 * (j + 1), :] \
                                    .rearrange("p (r w) -> p r w", r=8)
                                nc.tensor.matmul(
                                    out, wm[:, si, :], rhs,
                                    start=(si == 0), stop=(si == 5),
                                    tile_position=(0, 64 * j),
                                    skip_group_check=True)
                        idx = 0
                        for j, pl8 in enumerate((plA8, plB8)):
                            co = 64 * (1 - j)
                            for i, (di, dj, dk) in enumerate(dr_sets):
                                base = (r0 + di) * HP + dj
                                rhs = sub_ap(pl8[:], base, [(dk, 2), (1, CFL)])
                                nc.tensor.matmul(
                                    pc[:], wc[:, i, :, co:co + 128], rhs,
                                    start=(idx == 0), stop=(idx == 2 * ndr - 1),
                                    perf_mode=DRM, tile_position=(0, 0),
                                    skip_group_check=True)
                                idx += 1
                        # evacuation: copy main, add scaled corr, square
                        sl = dst_strip[:, CHW * cth:CHW * (cth + 1)]
                        slv = sl.rearrange("p (r w) -> p r w", w=W)
                        nc.scalar.activation(sl, pm[:], AF.Copy)
                        pcv = sub_ap(pc[:], 0, [(HP, 8), (1, W)])
                        nc.vector.scalar_tensor_tensor(
                            slv, pcv, 1.0 / CSC, slv, AO.mult, AO.add,
                            accum_out=sums_t[:, pcol * 7 + cth:pcol * 7 + cth + 1])
                        jk = hf.tile([128, CHW], F16, tag="jk", bufs=1)
                        nc.scalar.activation(
                            jk[:], sl, AF.Square,
                            accum_out=sumsq_t[:, pcol * 7 + cth:pcol * 7 + cth + 1])


                def conv_img_pair2(plA, plB, dst_strip, sums_t, sumsq_t, pcol):
                    """Baseline conv2: [W2hi;W2lo] x [s1;s1], 9 taps, K=128."""
                    plAr = plA[:, 0:HP * HP].rearrange("p (r w) -> p r w", w=HP)
                    plBr = plB[:, 0:HP * HP].rearrange("p (r w) -> p r w", w=HP)
                    for cth in range(NCH):
                        r0 = 8 * cth
                        pm = ps.tile([128, CHW], F32, tag="psm", bufs=4,
                                     name=f"p2m{cth & 1}")
                        for a in range(9):
                            di, dj = a // 3, a % 3
                            for j, plr in enumerate((plAr, plBr)):
                                rhs = plr[:, r0 + di:r0 + di + 8, dj:dj + W]
                                out = pm[64 * j:64 * (j + 1), :] \
                                    .rearrange("p (r w) -> p r w", r=8)
                                nc.tensor.matmul(
                                    out, w2as[:, 64 * a:64 * (a + 1)], rhs,
                                    start=(a == 0), stop=(a == 8),
                                    tile_position=(0, 64 * j),
                                    skip_group_check=True)
                        sl = dst_strip[:, CHW * cth:CHW * (cth + 1)]
                        nc.scalar.activation(
                            sl, pm[:], AF.Copy,
                            accum_out=sums_t[:, pcol * 7 + cth:pcol * 7 + cth + 1])
                        jk = hf.tile([128, CHW], F16, tag="jk", bufs=1)
                        nc.vector.scalar_tensor_tensor(
                            jk[:], sl, 1.0, sl, AO.bypass, AO.mult,
                            accum_out=sumsq_t[:, pcol * 7 + cth:pcol * 7 + cth + 1])

                # ================= phase A: conv1 =================
                y1s = []
                for p in range(NPAIR):
                    tt_, bp = p // 2, p % 2
                    iA = tt_ * 4 + bp * 2
                    pls, pl8s_ = [], []
                    for j in range(2):
                        i = iA + j
                        ta = plpool.tile([128, PPP], F16, tag="plf16")
                        nc.sync.dma_start(ta[:], pa[i])
                        t8 = plpool.tile([128, PPP], F8, tag="plf8")
                        nc.sync.dma_start(t8[:], pb[i])
                        pls.append(ta)
                        pl8s_.append(t8)
                    strip = yspool.tile([128, PIX], F32, tag="ys")
                    y1s.append(strip)
                    conv_img_pair(pls[0], pls[1], pl8s_[0], pl8s_[1],
                                  w1ms, w1cs, DR1_SETS, strip, sums1, sums1q, p)
                    if DBG:
                        nc.sync.dma_start(y1d[p], strip[:])

                # ---- stats1 allreduce
                cc1i = dramw.tile([128, 2], F32)
                cc1o = dramw.tile([128, 2], F32, addr_space="Shared")
                acc1 = tiny.tile([128, 2], F32, tag="acc")
                nc.vector.tensor_reduce(acc1[:, 0:1], sums1[:], AX.X, AO.add)
                nc.vector.tensor_reduce(acc1[:, 1:2], sums1q[:], AX.X, AO.add)
                nc.sync.dma_start(cc1i[:], acc1[:])
                if NO_CC:
                    nc.sync.dma_start(cc1o[:], cc1i[:])
                else:
                    nc.gpsimd.collective_compute(
                        "AllReduce", AO.add, ins=[cc1i[:]], outs=[cc1o[:]],
                        replica_groups=[list(range(NCORES))])
                g1 = tiny.tile([128, 2], F32, tag="acc")
                nc.sync.dma_start(g1[:], cc1o[:])

                def stats_block(g, beta, rgam, k1):
                    gr = tiny.tile([128, 2], F32, tag="acc")
                    nc.sync.dma_start(gr[0:64, :], g[64:128, :])
                    nc.sync.dma_start(gr[64:128, :], g[0:64, :])
                    tot = tiny.tile([128, 2], F32, tag="acc")
                    nc.vector.tensor_tensor(tot[:], g[:], gr[:], AO.add)
                    mean = tiny.tile([128, 1], F32, tag="t1")
                    nc.vector.tensor_scalar(mean[:], tot[:, 0:1], 1.0 / NG,
                                            None, AO.mult)
                    msq = tiny.tile([128, 1], F32, tag="t1")
                    nc.vector.tensor_scalar(msq[:], tot[:, 1:2], 1.0 / NG,
                                            None, AO.mult)
                    m2 = tiny.tile([128, 1], F32, tag="t1")
                    nc.vector.scalar_tensor_tensor(m2[:], mean[:], 1.0, mean[:],
                                                   AO.bypass, AO.mult)
                    var = tiny.tile([128, 1], F32, tag="t1")
                    nc.vector.tensor_tensor(var[:], msq[:], m2[:], AO.subtract)
                    std = tiny.tile([128, 1], F32, tag="t1")
                    nc.scalar.activation(std[:], var[:], AF.Sqrt, bias=epst[:])
                    rscv = tiny.tile([128, 1], F32, tag="t1")
                    nc.vector.tensor_tensor(rscv[:], std[:], rgam, AO.mult)
                    gamv = tiny.tile([128, 1], F32, tag="t1")
                    # gm = beta*rsc - mean
                    nc.vector.scalar_tensor_tensor(gamv[:], rscv[:], beta,
                                                   mean[:], AO.mult,
                                                   AO.subtract)
                    th = tiny.tile([128, 1], F32, tag="t1")
                    # th = std*K1 + mean, K1 = rga*(1 - alpha*beta)
                    nc.vector.scalar_tensor_tensor(th[:], std[:], k1,
                                                   mean[:], AO.mult, AO.add)
                    return th, gamv, rscv

                th1, gm1, _rsc1 = stats_block(
                    g1, cpars[:, 0:1], cpars[:, 2:3], cpars[:, 4:5])
                # wv1 bias: c1*gm1 so wv = c1*(q + gm1); Pneg = wv*(s-1) = -P
                gm1c1 = tiny.tile([128, 1], F32, tag="t1")
                nc.vector.tensor_scalar(gm1c1[:], gm1[:], c1, None, AO.mult)
                if DBG:
                    nc.sync.dma_start(vecd[:, 0:1], th1[:])
                    nc.sync.dma_start(vecd[:, 1:2], gm1[:])
                    nc.sync.dma_start(vecd[:, 4:5], acc1[:, 0:1])
                    nc.sync.dma_start(vecd[:, 5:6], acc1[:, 1:2])

                # ============ phase B + C: LIF1 + conv2 ============
                y2s = [None] * NPAIR
                for bp in range(2 if PHASES >= 2 else 0):
                    Pneg = [None] * NQ
                    for t in range(1, 5):
                        p = (t - 1) * 2 + bp
                        s1tq = []
                        for hq in range(NQ):
                            off = QL * hq
                            ysl = y1s[p][:, off:off + QL]
                            s1t = hf.tile([128, QL], F16, tag="s1t", bufs=3)
                            if t == 1:
                                nc.vector.tensor_scalar(s1t[:], ysl, th1[:],
                                                        None, AO.is_ge)
                            else:
                                # s = (y1 - th) >= Pneg  <=>  y1 + P >= th
                                nc.vector.scalar_tensor_tensor(
                                    s1t[:], ysl, th1[:], Pneg[hq][:],
                                    AO.subtract, AO.is_ge)
                            s1tq.append(s1t)
                            if DBG:
                                nc.gpsimd.dma_start(
                                    s1d[p, :, off:off + QL], s1t[:])
                            if t < 4:
                                mng = hf.tile([128, QL], F16, tag="mng", bufs=3)
                                nc.scalar.activation(mng[:], s1t[:],
                                                     AF.Identity,
                                                     bias=negone[:])
                                # v = q + gm1 = (y1 + gm1) - Pneg
                                v = hf.tile([128, QL], F32, tag="wvn", bufs=3)
                                if t == 1:
                                    nc.vector.tensor_scalar(
                                        v[:], ysl, gm1[:], None, AO.add)
                                else:
                                    nc.vector.scalar_tensor_tensor(
                                        v[:], ysl, gm1[:], Pneg[hq][:],
                                        AO.add, AO.subtract)
                                Pn = hf.tile([128, QL], F32, tag="pp", bufs=8)
                                # (c1*v) * (s-1) = -c1*v*(1-s) = -P'
                                nc.vector.scalar_tensor_tensor(
                                    Pn[:], v[:], c1, mng[:], AO.mult, AO.mult)
                                Pneg[hq] = Pn
                        # assemble spike planes [s1 ; s1] (duplicated)
                        sa_pair = []
                        for j in range(2):
                            sa = plpool.tile([128, PPP], F16, tag="plf16")
                            sar = sa[:, 0:HP * HP].rearrange(
                                "p (r w) -> p r w", w=HP)
                            for hq in range(NQ):
                                s1t = s1tq[hq]
                                src = s1t[64 * j:64 * (j + 1), :] \
                                    .rearrange("p (r w) -> p r w", w=W)
                                rl = 1 + 14 * hq
                                nc.sync.dma_start(
                                    sar[0:64, rl:rl + 14, 1:1 + W], src)
                                nc.sync.dma_start(
                                    sar[64:128, rl:rl + 14, 1:1 + W], src)
                            sa_pair.append(sa)
                        strip2 = yspool.tile([128, PIX], F32, tag="ys")
                        y2s[p] = strip2
                        conv_img_pair2(sa_pair[0], sa_pair[1],
                                       strip2, sums2, sums2q, p)
                        if DBG:
                            nc.sync.dma_start(y2d[p], strip2[:])

                # ---- stats2 allreduce
                cc2i = dramw.tile([128, 2], F32)
                cc2o = dramw.tile([128, 2], F32, addr_space="Shared")
                acc2 = tiny.tile([128, 2], F32, tag="acc")
                nc.vector.tensor_reduce(acc2[:, 0:1], sums2[:], AX.X, AO.add)
                nc.vector.tensor_reduce(acc2[:, 1:2], sums2q[:], AX.X, AO.add)
                nc.sync.dma_start(cc2i[:], acc2[:])
                if NO_CC:
                    nc.sync.dma_start(cc2o[:], cc2i[:])
                else:
                    nc.gpsimd.collective_compute(
                        "AllReduce", AO.add, ins=[cc2i[:]], outs=[cc2o[:]],
                        replica_groups=[list(range(NCORES))])
                g2 = tiny.tile([128, 2], F32, tag="acc")
                nc.sync.dma_start(g2[:], cc2o[:])
                th2, gm2, rsc2 = stats_block(
                    g2, cpars[:, 1:2], cpars[:, 3:4], cpars[:, 5:6])
                if DBG:
                    nc.sync.dma_start(vecd[:, 2:3], th2[:])
                    nc.sync.dma_start(vecd[:, 3:4], gm2[:])

                # ============ phase D: residual + LIF2 ============
                # t-outer emission: 8 independent (bp,hq) chains per step
                Pneg2 = {}
                for t in range(1 if PHASES >= 3 else 5, 5):
                    for bp in range(2):
                        p = (t - 1) * 2 + bp
                        iA = (t - 1) * 4 + bp * 2
                        for hq in range(NQ):
                            off = QL * hq
                            # wneg = Pneg - y2 on Pool, off the x path
                            wneg = hf.tile([128, QL], F32, tag="wvn", bufs=3)
                            if t == 1:
                                nc.gpsimd.tensor_tensor(
                                    wneg[:], zq[:], y2s[p][:, off:off + QL],
                                    AO.subtract)
                            else:
                                nc.gpsimd.tensor_tensor(
                                    wneg[:], Pneg2[bp, hq][:],
                                    y2s[p][:, off:off + QL], AO.subtract)
                            xs = hf.tile([128, QL], F32, tag="xs", bufs=4)
                            nc.sync.dma_start(
                                xs[:],
                                xin[iA:iA + 2, :, off:off + QL]
                                .rearrange("a p q -> (a p) q"))
                            # xsc = x*rsc2 in-place (Act)
                            nc.scalar.activation(xs[:], xs[:], AF.Copy,
                                                 scale=rsc2[:])
                            ot = hf.tile([128, QL], F16, tag="ot", bufs=3)
                            # s: (xsc - th) >= wneg  <=>  xsc + y2 + P >= th
                            nc.vector.scalar_tensor_tensor(
                                ot[:], xs[:], th2[:], wneg[:],
                                AO.subtract, AO.is_ge)
                            nc.scalar.dma_start(
                                outp[iA:iA + 2, :, off:off + QL]
                                .rearrange("a p q -> (a p) q"), ot[:])
                            if t < 4:
                                mng = hf.tile([128, QL], F16, tag="mng",
                                              bufs=3)
                                nc.scalar.activation(mng[:], ot[:],
                                                     AF.Identity,
                                                     bias=negone[:])
                                # u = q + gm2 = (xsc + gm2) - wneg, in-place
                                nc.vector.scalar_tensor_tensor(
                                    xs[:], xs[:], gm2[:], wneg[:],
                                    AO.add, AO.subtract)
                                Pn = hf.tile([128, QL], F32, tag="pp", bufs=8)
                                # (c2*u) * (s-1) = -c2*u*(1-s) = -P'
                                nc.vector.scalar_tensor_tensor(
                                    Pn[:], xs[:], c2, mng[:], AO.mult, AO.mult)
                                Pneg2[bp, hq] = Pn

    nc.compile()
    return nc, names


def _sigmoid(x):
    return 1.0 / (1.0 + np.exp(-float(x)))


def prepare(x, conv1_w, bn1_gamma, bn1_beta, lif1_w, conv2_w, bn2_gamma,
            bn2_beta, lif2_w):
    import ml_dtypes
    E4 = ml_dtypes.float8_e4m3

    x = np.ascontiguousarray(np.asarray(x, np.float32))
    conv1_w = np.asarray(conv1_w, np.float32)
    conv2_w = np.asarray(conv2_w, np.float32)

    a1 = _sigmoid(np.asarray(lif1_w).reshape(-1)[0])
    a2 = _sigmoid(np.asarray(lif2_w).reshape(-1)[0])

    key = (round(a1, 12), round(a2, 12))
    if key not in _prog_cache:
        _prog_cache[key] = _build(a1, a2)
    nc, names = _prog_cache[key]

    xh = x.astype(np.float16)
    xl = x - xh.astype(np.float32)

    # fp16 main planes: [xhi padded ; xhi shifted up one row]
    pa_np = np.zeros((T, B, 128, HP, HP), np.float16)
    pa_np[:, :, 0:64, 1:57, 1:57] = xh
    pa_np[:, :, 64:128, 0:56, 1:57] = xh
    # fp8 corr planes: [e4m3(x) ; e4m3(xl*2^11)], same (unshifted) layout
    pb_np = np.zeros((T, B, 128, HP, HP), E4)
    pb_np[:, :, 0:64, 1:57, 1:57] = x.astype(E4)
    pb_np[:, :, 64:128, 1:57, 1:57] = (xl * CSC).astype(E4)

    def pad_flat(arr):
        # [T,B,128,HP,HP] -> [T,B,128,PPP] with zero tail
        t_, b_, p_, _, _ = arr.shape
        out = np.zeros((t_, b_, p_, PPP), arr.dtype)
        out[..., :HP * HP] = arr.reshape(t_, b_, p_, HP * HP)
        return out

    pa_np = pad_flat(pa_np)
    pb_np = pad_flat(pb_np)

    w1h = conv1_w.astype(np.float16)
    w1l = conv1_w - w1h.astype(np.float32)
    w2h = conv2_w.astype(np.float16)
    w2l = conv2_w - w2h.astype(np.float32)

    def main_stack(wh):
        out = np.zeros((128, 6, 64), np.float16)
        for s, (di, dj) in enumerate(MAIN_SETS):
            if di == 0:
                out[0:64, s] = wh[:, :, 0, dj].T
                out[64:128, s] = wh[:, :, 1, dj].T
            else:
                out[0:64, s] = wh[:, :, 2, dj].T
        return out

    w1m_np = main_stack(w1h)

    def tapstack(wtop, wbot):
        out = np.zeros((128, 9 * 64), np.float16)
        for a in range(9):
            di, dj = a // 3, a % 3
            out[0:64, a * 64:(a + 1) * 64] = wtop[:, :, di, dj].T
            out[64:128, a * 64:(a + 1) * 64] = wbot[:, :, di, dj].T
        return out

    w2a_np = tapstack(w2h, w2l.astype(np.float16))

    # conv1 DR weights: k-tile block rows 0:64 multiply e4m3(x) -> W1l_s,
    # rows 64:128 multiply e4m3(xl*2^11) -> W1h8; active cols at 64:128.
    w1l_s = (w1l * CSC).astype(E4)
    w1h8 = conv1_w.astype(E4)
    w1c_np = np.zeros((128, 5, 2, 192), E4)
    dr1_taps = [((0, 0), (1, 0)), ((0, 1), (1, 1)), ((0, 2), (1, 2)),
                ((2, 0), (2, 2)), ((2, 1), None)]
    for i, (ta, tb) in enumerate(dr1_taps):
        for kk, tap in enumerate((ta, tb)):
            if tap is None:
                continue
            di, dj = tap
            w1c_np[0:64, i, kk, 64:128] = w1l_s[:, :, di, dj].astype(
                np.float32).T.astype(E4)
            w1c_np[64:128, i, kk, 64:128] = w1h8[:, :, di, dj].astype(
                np.float32).T.astype(E4)

    def dup(v):
        v = np.asarray(v, np.float32).reshape(64)
        return np.concatenate([v, v])

    cpar_np = np.zeros((128, 8), np.float32)
    cpar_np[:, 0] = dup(bn1_beta)
    cpar_np[:, 1] = dup(bn2_beta)
    cpar_np[:, 2] = 1.0 / dup(bn1_gamma)
    cpar_np[:, 3] = 1.0 / dup(bn2_gamma)
    rga1 = 1.0 / (a1 * dup(bn1_gamma))
    rga2 = 1.0 / (a2 * dup(bn2_gamma))
    cpar_np[:, 4] = rga1 * (1.0 - a1 * dup(bn1_beta))
    cpar_np[:, 5] = rga2 * (1.0 - a2 * dup(bn2_beta))

    in_maps = []
    for k in range(NCORES):
        sl = slice(4 * k, 4 * k + 4)
        pa_k = np.ascontiguousarray(pa_np[:, sl].reshape(NIMG, 128, PPP))
        pb_k = np.ascontiguousarray(pb_np[:, sl].reshape(NIMG, 128, PPP))
        xin_k = np.ascontiguousarray(x[:, sl].reshape(NIMG, 64, PIX))
        in_maps.append({
            names['pa']: pa_k,
            names['pb']: pb_k.view(np.uint8),
            names['xin']: xin_k,
            names['w1m']: w1m_np,
            names['w1c']: w1c_np.view(np.uint8),
            names['w2a']: w2a_np,
            names['cpar']: cpar_np,
        })

    return nc, names, in_maps


def kernel(**inputs):
    from concourse.bass_utils import run_bass_kernel_spmd
    nc, names, in_maps = prepare(**inputs)
    res = run_bass_kernel_spmd(nc, in_maps, core_ids=list(range(NCORES)))
    global LAST_RES, LAST_NAMES
    LAST_RES, LAST_NAMES = res, names
    out = np.empty((T, B, C, H, W), np.float32)
    for k in range(NCORES):
        o = np.asarray(res.results[k][names['outp']], np.float32)
        out[:, 4 * k:4 * k + 4] = o.reshape(T, BL, C, H, W)
    return out


if __name__ == "__main__":
    rng = np.random.default_rng(0)
    xs = rng.standard_normal((T, B, C, H, W)).astype(np.float32)
    w1 = (rng.standard_normal((64, 64, 3, 3)) * 0.05).astype(np.float32)
    w2 = (rng.standard_normal((64, 64, 3, 3)) * 0.05).astype(np.float32)
    o = kernel(xs, w1, np.ones(64, np.float32), np.zeros(64, np.float32),
               np.zeros(1, np.float32), w2, np.ones(64, np.float32),
               np.zeros(64, np.float32), np.zeros(1, np.float32))
    print("ran:", o.shape, float(o.mean()))


# revision 33
# speedup vs baseline: 1.6394x; 1.0658x over previous
"""Trainium2 Bass kernel for nn_BasicBlock (spiking CNN block).

Sharding: data-parallel over batch B across 8 NeuronCores (4 batch x 4
timesteps = 16 images per core); BN batch stats via tiny AllReduce.

Per core (vs the 27-pass baseline this runs 16 pass-equivalents):
- conv1 main (W1hi . xhi, fp16): taps row-paired via planes laid out as
  [xhi ; xhi shifted one row] so K=128 contracts two taps -> 6 passes.
- conv1 corr (W1hi . xlo + W1lo . xhi): fp8e4 DoubleRow matmuls (0.5
  cyc/row) over planes [e4m3(x) ; e4m3(xlo*2^11)], two taps per
  instruction, M=128 with half-zero weight columns so each image's
  correction lands on its own partition half of ONE corr PSUM bank.
  Streams flat at the padded pitch (58); pad columns hold garbage that
  the evacuation never reads. ~2.5 pass-equivalents.
- conv2: same structure on spike planes [s1 ; s1 shifted] (s1 exact in
  fp16 AND fp8): hi 6 passes + W2lo fp8-DR corr 1.5 pass-equivalents.
- Evacuation: Act copy (main psum) -> strip, DVE stt strip += corr*2^-11
  (accum_out -> BN sums), Act square (accum_out -> BN sumsq).
- PLIF scans in "q-space" with negated-state fusion: s and the carried
  state are scalar_tensor_tensor ops; reset mask via Act (s-1).
- Residual+LIF2 spread across Act (x*rsc2, mask), Pool (adds), DVE
  (compares + state); spikes written out as fp16 and cast on host.
"""
import sys
sys.path.insert(0, '/opt/trn_rl_repo')

import numpy as np

T, B, C, H, W = 4, 32, 64, 56, 56
NCORES = 8
BL = B // NCORES            # 4 local batch samples
NIMG = T * BL               # 16 images per core
HP = W + 2                  # 58
PPP = HP * HP + 4           # padded plane + flat-stream overrun guard (3368)
PIX = H * W                 # 3136
NCH = 7                     # conv chunks per image (8 rows each)
CHW = 8 * W                 # 448 compact chunk
CFL = 8 * HP                # 464 flat chunk span
NPAIR = 8                   # image pairs per core
EPS = 1e-5
NG = float((T * B) * PIX)   # 401408
QL = 14 * W                 # LIF quarter-strip length (784)
NQ = 4
CSC = 2048.0                # 2^11 fp8 correction scale

# (di, dj) slice per fp16 main set: di=0 -> taps (0,dj)+(1,dj) paired via
# the shifted upper half; di=2 -> tap (2,dj) solo (upper weights zero).
MAIN_SETS = [(0, 0), (0, 1), (0, 2), (2, 0), (2, 1), (2, 2)]
# conv1 fp8-DR sets: (di, dj, dk) -> k-tile1 at (di,dj), k-tile2 at +dk
# covering tap pairs ((0,j),(1,j))x3, ((2,0),(2,2)) [stride 2; odd k-tile
# strides crash the ifmap fetcher], ((2,1), zero)
DR1_SETS = [(0, 0, HP), (0, 1, HP), (0, 2, HP), (2, 0, 2), (2, 1, 2)]
# conv2 fp8-DR sets: k-tiles 2 rows apart cover taps (0,j),(1,j),(2,j),x0
DR2_SETS = [(0, 0, 2 * HP), (0, 1, 2 * HP), (0, 2, 2 * HP)]

_prog_cache = {}
DBG = False
NO_CC = False
PHASES = 3
TRACE = False
LAST_RES = None
LAST_NAMES = None
LAST_EXEC_NS = None


def _build(alpha1, alpha2):
    import concourse.mybir as mybir
    import concourse.tile as tile
    from concourse.ap import AP
    from concourse import bacc

    F32 = mybir.dt.float32
    F16 = mybir.dt.float16
    F8 = mybir.dt.float8e4
    AO = mybir.AluOpType
    AF = mybir.ActivationFunctionType
    AX = mybir.AxisListType
    DRM = mybir.MatmulPerfMode.DoubleRow

    c1 = 1.0 - alpha1
    c2 = 1.0 - alpha2

    def sub_ap(base, extra_off, dims):
        b = base.copy()
        return AP(b.tensor, b.offset + extra_off,
                  [list(b.ap[0])] + [list(d) for d in dims])

    nc = bacc.Bacc(None, target_bir_lowering=False)
    names = {}

    with tile.TileContext(nc) as tc:
        with tc.tile_pool(name="dram", bufs=1, space="DRAM") as dram:
            pa = dram.tile([NIMG, 128, PPP], F16, kind="ExternalInput")
            pb = dram.tile([NIMG, 128, PPP], F8, kind="ExternalInput")
            xin = dram.tile([NIMG, 64, PIX], F32, kind="ExternalInput")
            w1m = dram.tile([128, 6, 64], F16, kind="ExternalInput")
            w1c = dram.tile([128, 5, 2, 192], F8, kind="ExternalInput")
            w2m = dram.tile([128, 6, 64], F16, kind="ExternalInput")
            w2c = dram.tile([128, 3, 2, 192], F8, kind="ExternalInput")
            cpar = dram.tile([128, 8], F32, kind="ExternalInput")
            outp = dram.tile([NIMG, 64, PIX], F16, kind="ExternalOutput")
            names.update(pa=pa.name, pb=pb.name, xin=xin.name, w1m=w1m.name,
                         w1c=w1c.name, w2m=w2m.name, w2c=w2c.name,
                         cpar=cpar.name, outp=outp.name)
            if DBG:
                y1d = dram.tile([NPAIR, 128, PIX], F32, kind="ExternalOutput")
                y2d = dram.tile([NPAIR, 128, PIX], F32, kind="ExternalOutput")
                s1d = dram.tile([NPAIR, 128, PIX], F32, kind="ExternalOutput")
                vecd = dram.tile([128, 8], F32, kind="ExternalOutput")
                names.update(y1d=y1d.name, y2d=y2d.name, s1d=s1d.name,
                             vecd=vecd.name)

            with tc.tile_pool(name="dramw", bufs=1, space="DRAM") as dramw, \
                 tc.tile_pool(name="wsb", bufs=1) as wsb, \
                 tc.tile_pool(name="ys", bufs=8) as yspool, \
                 tc.tile_pool(name="plane", bufs=4) as plpool, \
                 tc.tile_pool(name="hfp", bufs=2) as hf, \
                 tc.tile_pool(name="tiny", bufs=17) as tiny, \
                 tc.tile_pool(name="ps", bufs=7, space="PSUM") as ps:

                # ---- static parameter loads
                w1ms = wsb.tile([128, 6, 64], F16, tag="w1m")
                nc.sync.dma_start(w1ms[:], w1m[:])
                w1cs = wsb.tile([128, 5, 2, 192], F8, tag="w1c")
                nc.sync.dma_start(w1cs[:], w1c[:])
                w2ms = wsb.tile([128, 6, 64], F16, tag="w2m")
                nc.sync.dma_start(w2ms[:], w2m[:])
                w2cs = wsb.tile([128, 3, 2, 192], F8, tag="w2c")
                nc.sync.dma_start(w2cs[:], w2c[:])
                cpars = wsb.tile([128, 8], F32, tag="cpar")
                nc.sync.dma_start(cpars[:], cpar[:])
                negone = wsb.tile([128, 1], F32, tag="negone")
                nc.vector.memset(negone[:], -1.0)
                zq = wsb.tile([128, QL], F32, tag="zq")
                nc.vector.memset(zq[:], 0.0)
                epst = wsb.tile([128, 1], F32, tag="epst")
                nc.vector.memset(epst[:], EPS)
                sums1 = wsb.tile([128, 56], F32, tag="sums1")
                sums1q = wsb.tile([128, 56], F32, tag="sums1q")
                sums2 = wsb.tile([128, 56], F32, tag="sums2")
                sums2q = wsb.tile([128, 56], F32, tag="sums2q")
                if PHASES < 2:
                    nc.vector.memset(sums2[:], 0.0)
                    nc.vector.memset(sums2q[:], 0.0)

                def conv_img_pair(plA, plB, plA8, plB8, wm, wc, dr_sets,
                                  dst_strip, sums_t, sumsq_t, pcol):
                    """One image pair: per chunk, 6 fp16 tap-paired matmuls
                    per image into a main psum + fp8 DoubleRow correction
                    instructions (both images) into one flat corr psum, then
                    the 3-op evacuation."""
                    plAr = plA[:, 0:HP * HP].rearrange("p (r w) -> p r w", w=HP)
                    plBr = plB[:, 0:HP * HP].rearrange("p (r w) -> p r w", w=HP)
                    ndr = len(dr_sets)
                    for cth in range(NCH):
                        r0 = 8 * cth
                        pm = ps.tile([128, CHW], F32, tag="psm", bufs=4,
                                     name=f"psm{cth & 1}")
                        pc = ps.tile([128, CFL], F32, tag="psc", bufs=3,
                                     name=f"psc{cth & 1}")
                        for si, (di, dj) in enumerate(MAIN_SETS):
                            for j, plr in enumerate((plAr, plBr)):
                                rhs = plr[:, r0 + di:r0 + di + 8, dj:dj + W]
                                out = pm[64 * j:64 * (j + 1), :] \
                                    .rearrange("p (r w) -> p r w", r=8)
                                nc.tensor.matmul(
                                    out, wm[:, si, :], rhs,
                                    start=(si == 0), stop=(si == 5),
                                    tile_position=(0, 64 * j),
                                    skip_group_check=True)
                        idx = 0
                        for j, pl8 in enumerate((plA8, plB8)):
                            co = 64 * (1 - j)
                            for i, (di, dj, dk) in enumerate(dr_sets):
                                base = (r0 + di) * HP + dj
                                rhs = sub_ap(pl8[:], base, [(dk, 2), (1, CFL)])
                                nc.tensor.matmul(
                                    pc[:], wc[:, i, :, co:co + 128], rhs,
                                    start=(idx == 0), stop=(idx == 2 * ndr - 1),
                                    perf_mode=DRM, tile_position=(0, 0),
                                    skip_group_check=True)
                                idx += 1
                        # evacuation: copy main, add scaled corr, square
                        sl = dst_strip[:, CHW * cth:CHW * (cth + 1)]
                        slv = sl.rearrange("p (r w) -> p r w", w=W)
                        nc.scalar.activation(sl, pm[:], AF.Copy)
                        pcv = sub_ap(pc[:], 0, [(HP, 8), (1, W)])
                        nc.vector.scalar_tensor_tensor(
                            slv, pcv, 1.0 / CSC, slv, AO.mult, AO.add,
                            accum_out=sums_t[:, pcol * 7 + cth:pcol * 7 + cth + 1])
                        jk = hf.tile([128, CHW], F16, tag="jk", bufs=1)
                        nc.scalar.activation(
                            jk[:], sl, AF.Square,
                            accum_out=sumsq_t[:, pcol * 7 + cth:pcol * 7 + cth + 1])


                # ================= phase A: conv1 =================
                y1s = []
                for p in range(NPAIR):
                    tt_, bp = p // 2, p % 2
                    iA = tt_ * 4 + bp * 2
                    pls, pl8s_ = [], []
                    for j in range(2):
                        i = iA + j
                        ta = plpool.tile([128, PPP], F16, tag="plf16")
                        nc.sync.dma_start(ta[:], pa[i])
                        t8 = plpool.tile([128, PPP], F8, tag="plf8")
                        nc.sync.dma_start(t8[:], pb[i])
                        pls.append(ta)
                        pl8s_.append(t8)
                    strip = yspool.tile([128, PIX], F32, tag="ys")
                    y1s.append(strip)
                    conv_img_pair(pls[0], pls[1], pl8s_[0], pl8s_[1],
                                  w1ms, w1cs, DR1_SETS, strip, sums1, sums1q, p)
                    if DBG:
                        nc.sync.dma_start(y1d[p], strip[:])

                # ---- stats1 allreduce
                cc1i = dramw.tile([128, 2], F32)
                cc1o = dramw.tile([128, 2], F32, addr_space="Shared")
                acc1 = tiny.tile([128, 2], F32, tag="acc")
                nc.vector.tensor_reduce(acc1[:, 0:1], sums1[:], AX.X, AO.add)
                nc.vector.tensor_reduce(acc1[:, 1:2], sums1q[:], AX.X, AO.add)
                nc.sync.dma_start(cc1i[:], acc1[:])
                if NO_CC:
                    nc.sync.dma_start(cc1o[:], cc1i[:])
                else:
                    nc.gpsimd.collective_compute(
                        "AllReduce", AO.add, ins=[cc1i[:]], outs=[cc1o[:]],
                        replica_groups=[list(range(NCORES))])
                g1 = tiny.tile([128, 2], F32, tag="acc")
                nc.sync.dma_start(g1[:], cc1o[:])

                def stats_block(g, beta, rgam, k1):
                    gr = tiny.tile([128, 2], F32, tag="acc")
                    nc.sync.dma_start(gr[0:64, :], g[64:128, :])
                    nc.sync.dma_start(gr[64:128, :], g[0:64, :])
                    tot = tiny.tile([128, 2], F32, tag="acc")
                    nc.vector.tensor_tensor(tot[:], g[:], gr[:], AO.add)
                    mean = tiny.tile([128, 1], F32, tag="t1")
                    nc.vector.tensor_scalar(mean[:], tot[:, 0:1], 1.0 / NG,
                                            None, AO.mult)
                    msq = tiny.tile([128, 1], F32, tag="t1")
                    nc.vector.tensor_scalar(msq[:], tot[:, 1:2], 1.0 / NG,
                                            None, AO.mult)
                    m2 = tiny.tile([128, 1], F32, tag="t1")
                    nc.vector.scalar_tensor_tensor(m2[:], mean[:], 1.0, mean[:],
                                                   AO.bypass, AO.mult)
                    var = tiny.tile([128, 1], F32, tag="t1")
                    nc.vector.tensor_tensor(var[:], msq[:], m2[:], AO.subtract)
                    std = tiny.tile([128, 1], F32, tag="t1")
                    nc.scalar.activation(std[:], var[:], AF.Sqrt, bias=epst[:])
                    rscv = tiny.tile([128, 1], F32, tag="t1")
                    nc.vector.tensor_tensor(rscv[:], std[:], rgam, AO.mult)
                    gamv = tiny.tile([128, 1], F32, tag="t1")
                    # gm = beta*rsc - mean
                    nc.vector.scalar_tensor_tensor(gamv[:], rscv[:], beta,
                                                   mean[:], AO.mult,
                                                   AO.subtract)
                    th = tiny.tile([128, 1], F32, tag="t1")
                    # th = std*K1 + mean, K1 = rga*(1 - alpha*beta)
                    nc.vector.scalar_tensor_tensor(th[:], std[:], k1,
                                                   mean[:], AO.mult, AO.add)
                    return th, gamv, rscv

                th1, gm1, _rsc1 = stats_block(
                    g1, cpars[:, 0:1], cpars[:, 2:3], cpars[:, 4:5])
                # wv1 bias: c1*gm1 so wv = c1*(q + gm1); Pneg = wv*(s-1) = -P
                gm1c1 = tiny.tile([128, 1], F32, tag="t1")
                nc.vector.tensor_scalar(gm1c1[:], gm1[:], c1, None, AO.mult)
                if DBG:
                    nc.sync.dma_start(vecd[:, 0:1], th1[:])
                    nc.sync.dma_start(vecd[:, 1:2], gm1[:])
                    nc.sync.dma_start(vecd[:, 4:5], acc1[:, 0:1])
                    nc.sync.dma_start(vecd[:, 5:6], acc1[:, 1:2])

                # ============ phase B + C: LIF1 + conv2 ============
                y2s = [None] * NPAIR
                for bp in range(2 if PHASES >= 2 else 0):
                    Pneg = [None] * NQ
                    for t in range(1, 5):
                        p = (t - 1) * 2 + bp
                        s1tq = []
                        for hq in range(NQ):
                            off = QL * hq
                            ysl = y1s[p][:, off:off + QL]
                            s1t8 = hf.tile([128, QL], F8, tag="s1t8",
                                           bufs=4)
                            if t == 1:
                                nc.vector.tensor_scalar(s1t8[:], ysl, th1[:],
                                                        None, AO.is_ge)
                            else:
                                # s = (y1 - th) >= Pneg  <=>  y1 + P >= th
                                nc.vector.scalar_tensor_tensor(
                                    s1t8[:], ysl, th1[:], Pneg[hq][:],
                                    AO.subtract, AO.is_ge)
                            s1tq.append(s1t8)
                            if DBG:
                                nc.gpsimd.dma_start(
                                    s1d[p, :, off:off + QL], s1t8[:])
                            if t < 4:
                                mng = hf.tile([128, QL], F16, tag="mng", bufs=2)
                                nc.scalar.activation(mng[:], s1t8[:],
                                                     AF.Identity,
                                                     bias=negone[:])
                                # v = q + gm1 = (y1 + gm1) - Pneg
                                v = hf.tile([128, QL], F32, tag="wvn", bufs=3)
                                if t == 1:
                                    nc.vector.tensor_scalar(
                                        v[:], ysl, gm1[:], None, AO.add)
                                else:
                                    nc.vector.scalar_tensor_tensor(
                                        v[:], ysl, gm1[:], Pneg[hq][:],
                                        AO.add, AO.subtract)
                                Pn = hf.tile([128, QL], F32, tag="pp", bufs=8)
                                # (c1*v) * (s-1) = -c1*v*(1-s) = -P'
                                nc.vector.scalar_tensor_tensor(
                                    Pn[:], v[:], c1, mng[:], AO.mult, AO.mult)
                                Pneg[hq] = Pn
                        # fp8 spike planes [s1 ; s1 shifted one row] serve
                        # both the f16-weight main passes and the fp8 DR corr
                        s8_pair = []
                        for j in range(2):
                            s8 = plpool.tile([128, PPP], F8, tag="plf8")
                            s8r = s8[:, 0:HP * HP].rearrange(
                                "p (r w) -> p r w", w=HP)
                            for hq in range(NQ):
                                s1t8 = s1tq[hq]
                                src8 = s1t8[64 * j:64 * (j + 1), :] \
                                    .rearrange("p (r w) -> p r w", w=W)
                                rl = 1 + 14 * hq
                                ru = 14 * hq
                                nc.sync.dma_start(
                                    s8r[0:64, rl:rl + 14, 1:1 + W], src8)
                                nc.scalar.dma_start(
                                    s8r[64:128, ru:ru + 14, 1:1 + W], src8)
                            s8_pair.append(s8)
                        strip2 = yspool.tile([128, PIX], F32, tag="ys")
                        y2s[p] = strip2
                        conv_img_pair(s8_pair[0], s8_pair[1], s8_pair[0],
                                      s8_pair[1], w2ms, w2cs, DR2_SETS,
                                      strip2, sums2, sums2q, p)
                        if DBG:
                            nc.sync.dma_start(y2d[p], strip2[:])

                # ---- stats2 allreduce
                cc2i = dramw.tile([128, 2], F32)
                cc2o = dramw.tile([128, 2], F32, addr_space="Shared")
                acc2 = tiny.tile([128, 2], F32, tag="acc")
                nc.vector.tensor_reduce(acc2[:, 0:1], sums2[:], AX.X, AO.add)
                nc.vector.tensor_reduce(acc2[:, 1:2], sums2q[:], AX.X, AO.add)
                nc.sync.dma_start(cc2i[:], acc2[:])
                if NO_CC:
                    nc.sync.dma_start(cc2o[:], cc2i[:])
                else:
                    nc.gpsimd.collective_compute(
                        "AllReduce", AO.add, ins=[cc2i[:]], outs=[cc2o[:]],
                        replica_groups=[list(range(NCORES))])
                g2 = tiny.tile([128, 2], F32, tag="acc")
                nc.sync.dma_start(g2[:], cc2o[:])
                th2, gm2, rsc2 = stats_block(
                    g2, cpars[:, 1:2], cpars[:, 3:4], cpars[:, 5:6])
                if DBG:
                    nc.sync.dma_start(vecd[:, 2:3], th2[:])
                    nc.sync.dma_start(vecd[:, 3:4], gm2[:])

                # ============ phase D: residual + LIF2 ============
                # t-outer emission: 8 independent (bp,hq) chains per step
                Pneg2 = {}
                for t in range(1 if PHASES >= 3 else 5, 5):
                    for bp in range(2):
                        p = (t - 1) * 2 + bp
                        iA = (t - 1) * 4 + bp * 2
                        for hq in range(NQ):
                            off = QL * hq
                            # wneg = Pneg - y2 on Pool, off the x path
                            wneg = hf.tile([128, QL], F32, tag="wvn", bufs=3)
                            if t == 1:
                                nc.gpsimd.tensor_tensor(
                                    wneg[:], zq[:], y2s[p][:, off:off + QL],
                                    AO.subtract)
                            else:
                                nc.gpsimd.tensor_tensor(
                                    wneg[:], Pneg2[bp, hq][:],
                                    y2s[p][:, off:off + QL], AO.subtract)
                            xs = hf.tile([128, QL], F32, tag="xs", bufs=4)
                            nc.sync.dma_start(
                                xs[:],
                                xin[iA:iA + 2, :, off:off + QL]
                                .rearrange("a p q -> (a p) q"))
                            # xsc = x*rsc2 in-place (Act)
                            nc.scalar.activation(xs[:], xs[:], AF.Copy,
                                                 scale=rsc2[:])
                            ot = hf.tile([128, QL], F16, tag="ot", bufs=3)
                            # s: (xsc - th) >= wneg  <=>  xsc + y2 + P >= th
                            nc.vector.scalar_tensor_tensor(
                                ot[:], xs[:], th2[:], wneg[:],
                                AO.subtract, AO.is_ge)
                            nc.scalar.dma_start(
                                outp[iA:iA + 2, :, off:off + QL]
                                .rearrange("a p q -> (a p) q"), ot[:])
                            if t < 4:
                                mng = hf.tile([128, QL], F16, tag="mng",
                                              bufs=2)
                                nc.scalar.activation(mng[:], ot[:],
                                                     AF.Identity,
                                                     bias=negone[:])
                                # u = q + gm2 = (xsc + gm2) - wneg, in-place
                                nc.vector.scalar_tensor_tensor(
                                    xs[:], xs[:], gm2[:], wneg[:],
                                    AO.add, AO.subtract)
                                Pn = hf.tile([128, QL], F32, tag="pp", bufs=8)
                                # (c2*u) * (s-1) = -c2*u*(1-s) = -P'
                                nc.vector.scalar_tensor_tensor(
                                    Pn[:], xs[:], c2, mng[:], AO.mult, AO.mult)
                                Pneg2[bp, hq] = Pn

    nc.compile()
    return nc, names


def _sigmoid(x):
    return 1.0 / (1.0 + np.exp(-float(x)))


def prepare(x, conv1_w, bn1_gamma, bn1_beta, lif1_w, conv2_w, bn2_gamma,
            bn2_beta, lif2_w):
    import ml_dtypes
    E4 = ml_dtypes.float8_e4m3

    x = np.ascontiguousarray(np.asarray(x, np.float32))
    conv1_w = np.asarray(conv1_w, np.float32)
    conv2_w = np.asarray(conv2_w, np.float32)

    a1 = _sigmoid(np.asarray(lif1_w).reshape(-1)[0])
    a2 = _sigmoid(np.asarray(lif2_w).reshape(-1)[0])

    key = (round(a1, 12), round(a2, 12))
    if key not in _prog_cache:
        _prog_cache[key] = _build(a1, a2)
    nc, names = _prog_cache[key]

    xh = x.astype(np.float16)
    xl = x - xh.astype(np.float32)

    # fp16 main planes: [xhi padded ; xhi shifted up one row]
    pa_np = np.zeros((T, B, 128, HP, HP), np.float16)
    pa_np[:, :, 0:64, 1:57, 1:57] = xh
    pa_np[:, :, 64:128, 0:56, 1:57] = xh
    # fp8 corr planes: [e4m3(x) ; e4m3(xl*2^11)], same (unshifted) layout
    pb_np = np.zeros((T, B, 128, HP, HP), E4)
    pb_np[:, :, 0:64, 1:57, 1:57] = x.astype(E4)
    pb_np[:, :, 64:128, 1:57, 1:57] = (xl * CSC).astype(E4)

    def pad_flat(arr):
        # [T,B,128,HP,HP] -> [T,B,128,PPP] with zero tail
        t_, b_, p_, _, _ = arr.shape
        out = np.zeros((t_, b_, p_, PPP), arr.dtype)
        out[..., :HP * HP] = arr.reshape(t_, b_, p_, HP * HP)
        return out

    pa_np = pad_flat(pa_np)
    pb_np = pad_flat(pb_np)

    w1h = conv1_w.astype(np.float16)
    w1l = conv1_w - w1h.astype(np.float32)
    w2h = conv2_w.astype(np.float16)
    w2l = conv2_w - w2h.astype(np.float32)

    def main_stack(wh):
        out = np.zeros((128, 6, 64), np.float16)
        for s, (di, dj) in enumerate(MAIN_SETS):
            if di == 0:
                out[0:64, s] = wh[:, :, 0, dj].T
                out[64:128, s] = wh[:, :, 1, dj].T
            else:
                out[0:64, s] = wh[:, :, 2, dj].T
        return out

    w1m_np = main_stack(w1h)

    w2m_np = main_stack(w2h)
    w2l_s = (w2l * CSC).astype(E4)
    w2c_np = np.zeros((128, 3, 2, 192), E4)
    for dj in range(3):
        w2c_np[0:64, dj, 0, 64:128] = w2l_s[:, :, 0, dj].astype(
            np.float32).T.astype(E4)
        w2c_np[64:128, dj, 0, 64:128] = w2l_s[:, :, 1, dj].astype(
            np.float32).T.astype(E4)
        w2c_np[0:64, dj, 1, 64:128] = w2l_s[:, :, 2, dj].astype(
            np.float32).T.astype(E4)

    # conv1 DR weights: k-tile block rows 0:64 multiply e4m3(x) -> W1l_s,
    # rows 64:128 multiply e4m3(xl*2^11) -> W1h8; active cols at 64:128.
    w1l_s = (w1l * CSC).astype(E4)
    w1h8 = conv1_w.astype(E4)
    w1c_np = np.zeros((128, 5, 2, 192), E4)
    dr1_taps = [((0, 0), (1, 0)), ((0, 1), (1, 1)), ((0, 2), (1, 2)),
                ((2, 0), (2, 2)), ((2, 1), None)]
    for i, (ta, tb) in enumerate(dr1_taps):
        for kk, tap in enumerate((ta, tb)):
            if tap is None:
                continue
            di, dj = tap
            w1c_np[0:64, i, kk, 64:128] = w1l_s[:, :, di, dj].astype(
                np.float32).T.astype(E4)
            w1c_np[64:128, i, kk, 64:128] = w1h8[:, :, di, dj].astype(
                np.float32).T.astype(E4)

    def dup(v):
        v = np.asarray(v, np.float32).reshape(64)
        return np.concatenate([v, v])

    cpar_np = np.zeros((128, 8), np.float32)
    cpar_np[:, 0] = dup(bn1_beta)
    cpar_np[:, 1] = dup(bn2_beta)
    cpar_np[:, 2] = 1.0 / dup(bn1_gamma)
    cpar_np[:, 3] = 1.0 / dup(bn2_gamma)
    rga1 = 1.0 / (a1 * dup(bn1_gamma))
    rga2 = 1.0 / (a2 * dup(bn2_gamma))
    cpar_np[:, 4] = rga1 * (1.0 - a1 * dup(bn1_beta))
    cpar_np[:, 5] = rga2 * (1.0 - a2 * dup(bn2_beta))

    in_maps = []
    for k in range(NCORES):
        sl = slice(4 * k, 4 * k + 4)
        pa_k = np.ascontiguousarray(pa_np[:, sl].reshape(NIMG, 128, PPP))
        pb_k = np.ascontiguousarray(pb_np[:, sl].reshape(NIMG, 128, PPP))
        xin_k = np.ascontiguousarray(x[:, sl].reshape(NIMG, 64, PIX))
        in_maps.append({
            names['pa']: pa_k,
            names['pb']: pb_k.view(np.uint8),
            names['xin']: xin_k,
            names['w1m']: w1m_np,
            names['w1c']: w1c_np.view(np.uint8),
            names['w2m']: w2m_np,
            names['w2c']: w2c_np.view(np.uint8),
            names['cpar']: cpar_np,
        })

    return nc, names, in_maps


def kernel(**inputs):
    from concourse.bass_utils import run_bass_kernel_spmd
    nc, names, in_maps = prepare(**inputs)
    res = run_bass_kernel_spmd(nc, in_maps, core_ids=list(range(NCORES)))
    global LAST_RES, LAST_NAMES
    LAST_RES, LAST_NAMES = res, names
    out = np.empty((T, B, C, H, W), np.float32)
    for k in range(NCORES):
        o = np.asarray(res.results[k][names['outp']], np.float32)
        out[:, 4 * k:4 * k + 4] = o.reshape(T, BL, C, H, W)
    return out


if __name__ == "__main__":
    rng = np.random.default_rng(0)
    xs = rng.standard_normal((T, B, C, H, W)).astype(np.float32)
    w1 = (rng.standard_normal((64, 64, 3, 3)) * 0.05).astype(np.float32)
    w2 = (rng.standard_normal((64, 64, 3, 3)) * 0.05).astype(np.float32)
    o = kernel(xs, w1, np.ones(64, np.float32), np.zeros(64, np.float32),
               np.zeros(1, np.float32), w2, np.ones(64, np.float32),
               np.zeros(64, np.float32), np.zeros(1, np.float32))
    print("ran:", o.shape, float(o.mean()))


# revision 34
# speedup vs baseline: 1.6542x; 1.0091x over previous
"""Trainium2 Bass kernel for nn_BasicBlock (spiking CNN block).

Sharding: data-parallel over batch B across 8 NeuronCores (4 batch x 4
timesteps = 16 images per core); BN batch stats via tiny AllReduce.

Per core (vs the 27-pass baseline this runs 16 pass-equivalents):
- conv1 main (W1hi . xhi, fp16): taps row-paired via planes laid out as
  [xhi ; xhi shifted one row] so K=128 contracts two taps -> 6 passes.
- conv1 corr (W1hi . xlo + W1lo . xhi): fp8e4 DoubleRow matmuls (0.5
  cyc/row) over planes [e4m3(x) ; e4m3(xlo*2^11)], two taps per
  instruction, M=128 with half-zero weight columns so each image's
  correction lands on its own partition half of ONE corr PSUM bank.
  Streams flat at the padded pitch (58); pad columns hold garbage that
  the evacuation never reads. ~2.5 pass-equivalents.
- conv2: same structure on spike planes [s1 ; s1 shifted] (s1 exact in
  fp16 AND fp8): hi 6 passes + W2lo fp8-DR corr 1.5 pass-equivalents.
- Evacuation: Act copy (main psum) -> strip, DVE stt strip += corr*2^-11
  (accum_out -> BN sums), Act square (accum_out -> BN sumsq).
- PLIF scans in "q-space" with negated-state fusion: s and the carried
  state are scalar_tensor_tensor ops; reset mask via Act (s-1).
- Residual+LIF2 spread across Act (x*rsc2, mask), Pool (adds), DVE
  (compares + state); spikes written out as fp16 and cast on host.
"""
import sys
sys.path.insert(0, '/opt/trn_rl_repo')

import numpy as np

T, B, C, H, W = 4, 32, 64, 56, 56
NCORES = 8
BL = B // NCORES            # 4 local batch samples
NIMG = T * BL               # 16 images per core
HP = W + 2                  # 58
PPP = HP * HP + 4           # padded plane + flat-stream overrun guard (3368)
PIX = H * W                 # 3136
NCH = 7                     # conv chunks per image (8 rows each)
CHW = 8 * W                 # 448 compact chunk
CFL = 8 * HP                # 464 flat chunk span
NPAIR = 8                   # image pairs per core
EPS = 1e-5
NG = float((T * B) * PIX)   # 401408
QL = 14 * W                 # LIF quarter-strip length (784)
NQ = 4
CSC = 2048.0                # 2^11 fp8 correction scale

# (di, dj) slice per fp16 main set: di=0 -> taps (0,dj)+(1,dj) paired via
# the shifted upper half; di=2 -> tap (2,dj) solo (upper weights zero).
MAIN_SETS = [(0, 0), (0, 1), (0, 2), (2, 0), (2, 1), (2, 2)]
# conv1 fp8-DR sets: (di, dj, dk) -> k-tile1 at (di,dj), k-tile2 at +dk
# covering tap pairs ((0,j),(1,j))x3, ((2,0),(2,2)) [stride 2; odd k-tile
# strides crash the ifmap fetcher], ((2,1), zero)
DR1_SETS = [(0, 0, HP), (0, 1, HP), (0, 2, HP), (2, 0, 2), (2, 1, 2)]
# conv2 fp8-DR sets: k-tiles 2 rows apart cover taps (0,j),(1,j),(2,j),x0
DR2_SETS = [(0, 0, 2 * HP), (0, 1, 2 * HP), (0, 2, 2 * HP)]

_prog_cache = {}
DBG = False
NO_CC = False
PHASES = 3
TRACE = False
LAST_RES = None
LAST_NAMES = None
LAST_EXEC_NS = None


def _build(alpha1, alpha2):
    import concourse.mybir as mybir
    import concourse.tile as tile
    from concourse.ap import AP
    from concourse import bacc

    F32 = mybir.dt.float32
    F16 = mybir.dt.float16
    F8 = mybir.dt.float8e4
    AO = mybir.AluOpType
    AF = mybir.ActivationFunctionType
    AX = mybir.AxisListType
    DRM = mybir.MatmulPerfMode.DoubleRow

    c1 = 1.0 - alpha1
    c2 = 1.0 - alpha2

    def sub_ap(base, extra_off, dims):
        b = base.copy()
        return AP(b.tensor, b.offset + extra_off,
                  [list(b.ap[0])] + [list(d) for d in dims])

    nc = bacc.Bacc(None, target_bir_lowering=False)
    names = {}

    with tile.TileContext(nc) as tc:
        with tc.tile_pool(name="dram", bufs=1, space="DRAM") as dram:
            pa = dram.tile([NIMG, 128, PPP], F16, kind="ExternalInput")
            pb = dram.tile([NIMG, 128, PPP], F8, kind="ExternalInput")
            xin = dram.tile([NIMG, 64, PIX], F32, kind="ExternalInput")
            w1m = dram.tile([128, 6, 64], F16, kind="ExternalInput")
            w1c = dram.tile([128, 5, 2, 192], F8, kind="ExternalInput")
            w2m = dram.tile([128, 6, 64], F16, kind="ExternalInput")
            w2c = dram.tile([128, 3, 2, 192], F8, kind="ExternalInput")
            cpar = dram.tile([128, 8], F32, kind="ExternalInput")
            outp = dram.tile([NIMG, 64, PIX], F16, kind="ExternalOutput")
            names.update(pa=pa.name, pb=pb.name, xin=xin.name, w1m=w1m.name,
                         w1c=w1c.name, w2m=w2m.name, w2c=w2c.name,
                         cpar=cpar.name, outp=outp.name)
            if DBG:
                y1d = dram.tile([NPAIR, 128, PIX], F32, kind="ExternalOutput")
                y2d = dram.tile([NPAIR, 128, PIX], F32, kind="ExternalOutput")
                s1d = dram.tile([NPAIR, 128, PIX], F32, kind="ExternalOutput")
                vecd = dram.tile([128, 8], F32, kind="ExternalOutput")
                names.update(y1d=y1d.name, y2d=y2d.name, s1d=s1d.name,
                             vecd=vecd.name)

            with tc.tile_pool(name="dramw", bufs=1, space="DRAM") as dramw, \
                 tc.tile_pool(name="wsb", bufs=1) as wsb, \
                 tc.tile_pool(name="ys", bufs=8) as yspool, \
                 tc.tile_pool(name="plane", bufs=4) as plpool, \
                 tc.tile_pool(name="hfp", bufs=2) as hf, \
                 tc.tile_pool(name="tiny", bufs=17) as tiny, \
                 tc.tile_pool(name="ps", bufs=7, space="PSUM") as ps:

                # ---- static parameter loads
                w1ms = wsb.tile([128, 6, 64], F16, tag="w1m")
                nc.sync.dma_start(w1ms[:], w1m[:])
                w1cs = wsb.tile([128, 5, 2, 192], F8, tag="w1c")
                nc.sync.dma_start(w1cs[:], w1c[:])
                w2ms = wsb.tile([128, 6, 64], F16, tag="w2m")
                nc.sync.dma_start(w2ms[:], w2m[:])
                w2cs = wsb.tile([128, 3, 2, 192], F8, tag="w2c")
                nc.sync.dma_start(w2cs[:], w2c[:])
                cpars = wsb.tile([128, 8], F32, tag="cpar")
                nc.sync.dma_start(cpars[:], cpar[:])
                negone = wsb.tile([128, 1], F32, tag="negone")
                nc.vector.memset(negone[:], -1.0)
                zq = wsb.tile([128, QL], F32, tag="zq")
                nc.vector.memset(zq[:], 0.0)
                epst = wsb.tile([128, 1], F32, tag="epst")
                nc.vector.memset(epst[:], EPS)
                sums1 = wsb.tile([128, 56], F32, tag="sums1")
                sums1q = wsb.tile([128, 56], F32, tag="sums1q")
                sums2 = wsb.tile([128, 56], F32, tag="sums2")
                sums2q = wsb.tile([128, 56], F32, tag="sums2q")
                if PHASES < 2:
                    nc.vector.memset(sums2[:], 0.0)
                    nc.vector.memset(sums2q[:], 0.0)

                def conv_img_pair(plA, plB, plA8, plB8, wm, wc, dr_sets,
                                  dst_strip, sums_t, sumsq_t, pcol):
                    """One image pair: per chunk, 6 fp16 tap-paired matmuls
                    per image into a main psum + fp8 DoubleRow correction
                    instructions (both images) into one flat corr psum, then
                    the 3-op evacuation."""
                    plAr = plA[:, 0:HP * HP].rearrange("p (r w) -> p r w", w=HP)
                    plBr = plB[:, 0:HP * HP].rearrange("p (r w) -> p r w", w=HP)
                    ndr = len(dr_sets)
                    for cth in range(NCH):
                        r0 = 8 * cth
                        pm = ps.tile([128, CHW], F32, tag="psm", bufs=4,
                                     name=f"psm{cth & 1}")
                        pc = ps.tile([128, CFL], F32, tag="psc", bufs=3,
                                     name=f"psc{cth & 1}")
                        for si, (di, dj) in enumerate(MAIN_SETS):
                            for j, plr in enumerate((plAr, plBr)):
                                rhs = plr[:, r0 + di:r0 + di + 8, dj:dj + W]
                                out = pm[64 * j:64 * (j + 1), :] \
                                    .rearrange("p (r w) -> p r w", r=8)
                                nc.tensor.matmul(
                                    out, wm[:, si, :], rhs,
                                    start=(si == 0), stop=(si == 5),
                                    tile_position=(0, 64 * j),
                                    skip_group_check=True)
                        idx = 0
                        for j, pl8 in enumerate((plA8, plB8)):
                            co = 64 * (1 - j)
                            for i, (di, dj, dk) in enumerate(dr_sets):
                                base = (r0 + di) * HP + dj
                                rhs = sub_ap(pl8[:], base, [(dk, 2), (1, CFL)])
                                nc.tensor.matmul(
                                    pc[:], wc[:, i, :, co:co + 128], rhs,
                                    start=(idx == 0), stop=(idx == 2 * ndr - 1),
                                    perf_mode=DRM, tile_position=(0, 0),
                                    skip_group_check=True)
                                idx += 1
                        # evacuation: copy main, add scaled corr, square
                        sl = dst_strip[:, CHW * cth:CHW * (cth + 1)]
                        slv = sl.rearrange("p (r w) -> p r w", w=W)
                        nc.scalar.activation(sl, pm[:], AF.Copy)
                        pcv = sub_ap(pc[:], 0, [(HP, 8), (1, W)])
                        nc.vector.scalar_tensor_tensor(
                            slv, pcv, 1.0 / CSC, slv, AO.mult, AO.add,
                            accum_out=sums_t[:, pcol * 7 + cth:pcol * 7 + cth + 1])
                        jk = hf.tile([128, CHW], F16, tag="jk", bufs=1)
                        nc.scalar.activation(
                            jk[:], sl, AF.Square,
                            accum_out=sumsq_t[:, pcol * 7 + cth:pcol * 7 + cth + 1])


                # ================= phase A: conv1 =================
                y1s = []
                for p in range(NPAIR):
                    tt_, bp = p // 2, p % 2
                    iA = tt_ * 4 + bp * 2
                    pls, pl8s_ = [], []
                    for j in range(2):
                        i = iA + j
                        ta = plpool.tile([128, PPP], F16, tag="plf16")
                        t8 = plpool.tile([128, PPP], F8, tag="plf8")
                        if p == 0:
                            hh = 30 * HP
                            nc.sync.dma_start(ta[:, 0:hh], pa[i, :, 0:hh])
                            nc.sync.dma_start(ta[:, hh:PPP], pa[i, :, hh:PPP])
                            nc.scalar.dma_start(t8[:, 0:hh], pb[i, :, 0:hh])
                            nc.scalar.dma_start(t8[:, hh:PPP], pb[i, :, hh:PPP])
                        else:
                            nc.sync.dma_start(ta[:], pa[i])
                            nc.scalar.dma_start(t8[:], pb[i])
                        pls.append(ta)
                        pl8s_.append(t8)
                    strip = yspool.tile([128, PIX], F32, tag="ys")
                    y1s.append(strip)
                    conv_img_pair(pls[0], pls[1], pl8s_[0], pl8s_[1],
                                  w1ms, w1cs, DR1_SETS, strip, sums1, sums1q, p)
                    if DBG:
                        nc.sync.dma_start(y1d[p], strip[:])

                # ---- stats1 allreduce
                cc1i = dramw.tile([128, 2], F32)
                cc1o = dramw.tile([128, 2], F32, addr_space="Shared")
                acc1 = tiny.tile([128, 2], F32, tag="acc")
                nc.vector.tensor_reduce(acc1[:, 0:1], sums1[:], AX.X, AO.add)
                nc.vector.tensor_reduce(acc1[:, 1:2], sums1q[:], AX.X, AO.add)
                nc.sync.dma_start(cc1i[:], acc1[:])
                if NO_CC:
                    nc.sync.dma_start(cc1o[:], cc1i[:])
                else:
                    nc.gpsimd.collective_compute(
                        "AllReduce", AO.add, ins=[cc1i[:]], outs=[cc1o[:]],
                        replica_groups=[list(range(NCORES))])

                def stats_block(cco, beta, rgam, k1):
                    g = tiny.tile([128, 2], F32, tag="acc")
                    nc.sync.dma_start(g[:], cco[:])
                    gr = tiny.tile([128, 2], F32, tag="acc")
                    nc.scalar.dma_start(gr[0:64, :], cco[64:128, :])
                    nc.scalar.dma_start(gr[64:128, :], cco[0:64, :])
                    tot = tiny.tile([128, 2], F32, tag="acc")
                    nc.vector.tensor_tensor(tot[:], g[:], gr[:], AO.add)
                    mean = tiny.tile([128, 1], F32, tag="t1")
                    nc.vector.tensor_scalar(mean[:], tot[:, 0:1], 1.0 / NG,
                                            None, AO.mult)
                    msq = tiny.tile([128, 1], F32, tag="t1")
                    nc.vector.tensor_scalar(msq[:], tot[:, 1:2], 1.0 / NG,
                                            None, AO.mult)
                    m2 = tiny.tile([128, 1], F32, tag="t1")
                    nc.vector.scalar_tensor_tensor(m2[:], mean[:], 1.0, mean[:],
                                                   AO.bypass, AO.mult)
                    var = tiny.tile([128, 1], F32, tag="t1")
                    nc.vector.tensor_tensor(var[:], msq[:], m2[:], AO.subtract)
                    std = tiny.tile([128, 1], F32, tag="t1")
                    nc.scalar.activation(std[:], var[:], AF.Sqrt, bias=epst[:])
                    rscv = tiny.tile([128, 1], F32, tag="t1")
                    nc.vector.tensor_tensor(rscv[:], std[:], rgam, AO.mult)
                    gamv = tiny.tile([128, 1], F32, tag="t1")
                    # gm = beta*rsc - mean
                    nc.vector.scalar_tensor_tensor(gamv[:], rscv[:], beta,
                                                   mean[:], AO.mult,
                                                   AO.subtract)
                    th = tiny.tile([128, 1], F32, tag="t1")
                    # th = std*K1 + mean, K1 = rga*(1 - alpha*beta)
                    nc.vector.scalar_tensor_tensor(th[:], std[:], k1,
                                                   mean[:], AO.mult, AO.add)
                    return th, gamv, rscv

                th1, gm1, _rsc1 = stats_block(
                    cc1o, cpars[:, 0:1], cpars[:, 2:3], cpars[:, 4:5])
                # wv1 bias: c1*gm1 so wv = c1*(q + gm1); Pneg = wv*(s-1) = -P
                gm1c1 = tiny.tile([128, 1], F32, tag="t1")
                nc.vector.tensor_scalar(gm1c1[:], gm1[:], c1, None, AO.mult)
                if DBG:
                    nc.sync.dma_start(vecd[:, 0:1], th1[:])
                    nc.sync.dma_start(vecd[:, 1:2], gm1[:])
                    nc.sync.dma_start(vecd[:, 4:5], acc1[:, 0:1])
                    nc.sync.dma_start(vecd[:, 5:6], acc1[:, 1:2])

                # ============ phase B + C: LIF1 + conv2 ============
                y2s = [None] * NPAIR
                for bp in range(2 if PHASES >= 2 else 0):
                    Pneg = [None] * NQ
                    for t in range(1, 5):
                        p = (t - 1) * 2 + bp
                        s1tq = []
                        for hq in range(NQ):
                            off = QL * hq
                            ysl = y1s[p][:, off:off + QL]
                            s1t8 = hf.tile([128, QL], F8, tag="s1t8",
                                           bufs=4)
                            if t == 1:
                                nc.vector.tensor_scalar(s1t8[:], ysl, th1[:],
                                                        None, AO.is_ge)
                            else:
                                # s = (y1 - th) >= Pneg  <=>  y1 + P >= th
                                nc.vector.scalar_tensor_tensor(
                                    s1t8[:], ysl, th1[:], Pneg[hq][:],
                                    AO.subtract, AO.is_ge)
                            s1tq.append(s1t8)
                            if DBG:
                                nc.gpsimd.dma_start(
                                    s1d[p, :, off:off + QL], s1t8[:])
                            if t < 4:
                                mng = hf.tile([128, QL], F16, tag="mng", bufs=2)
                                nc.scalar.activation(mng[:], s1t8[:],
                                                     AF.Identity,
                                                     bias=negone[:])
                                # v = q + gm1 = (y1 + gm1) - Pneg
                                v = hf.tile([128, QL], F32, tag="wvn", bufs=3)
                                if t == 1:
                                    nc.vector.tensor_scalar(
                                        v[:], ysl, gm1[:], None, AO.add)
                                else:
                                    nc.vector.scalar_tensor_tensor(
                                        v[:], ysl, gm1[:], Pneg[hq][:],
                                        AO.add, AO.subtract)
                                Pn = hf.tile([128, QL], F32, tag="pp", bufs=8)
                                # (c1*v) * (s-1) = -c1*v*(1-s) = -P'
                                nc.vector.scalar_tensor_tensor(
                                    Pn[:], v[:], c1, mng[:], AO.mult, AO.mult)
                                Pneg[hq] = Pn
                        # fp8 spike planes [s1 ; s1 shifted one row] serve
                        # both the f16-weight main passes and the fp8 DR corr
                        s8_pair = []
                        for j in range(2):
                            s8 = plpool.tile([128, PPP], F8, tag="plf8")
                            s8r = s8[:, 0:HP * HP].rearrange(
                                "p (r w) -> p r w", w=HP)
                            for hq in range(NQ):
                                s1t8 = s1tq[hq]
                                src8 = s1t8[64 * j:64 * (j + 1), :] \
                                    .rearrange("p (r w) -> p r w", w=W)
                                rl = 1 + 14 * hq
                                ru = 14 * hq
                                nc.sync.dma_start(
                                    s8r[0:64, rl:rl + 14, 1:1 + W], src8)
                                nc.scalar.dma_start(
                                    s8r[64:128, ru:ru + 14, 1:1 + W], src8)
                            s8_pair.append(s8)
                        strip2 = yspool.tile([128, PIX], F32, tag="ys")
                        y2s[p] = strip2
                        conv_img_pair(s8_pair[0], s8_pair[1], s8_pair[0],
                                      s8_pair[1], w2ms, w2cs, DR2_SETS,
                                      strip2, sums2, sums2q, p)
                        if DBG:
                            nc.sync.dma_start(y2d[p], strip2[:])

                # ---- stats2 allreduce
                cc2i = dramw.tile([128, 2], F32)
                cc2o = dramw.tile([128, 2], F32, addr_space="Shared")
                acc2 = tiny.tile([128, 2], F32, tag="acc")
                nc.vector.tensor_reduce(acc2[:, 0:1], sums2[:], AX.X, AO.add)
                nc.vector.tensor_reduce(acc2[:, 1:2], sums2q[:], AX.X, AO.add)
                nc.sync.dma_start(cc2i[:], acc2[:])
                if NO_CC:
                    nc.sync.dma_start(cc2o[:], cc2i[:])
                else:
                    nc.gpsimd.collective_compute(
                        "AllReduce", AO.add, ins=[cc2i[:]], outs=[cc2o[:]],
                        replica_groups=[list(range(NCORES))])
                th2, gm2, rsc2 = stats_block(
                    cc2o, cpars[:, 1:2], cpars[:, 3:4], cpars[:, 5:6])
                if DBG:
                    nc.sync.dma_start(vecd[:, 2:3], th2[:])
                    nc.sync.dma_start(vecd[:, 3:4], gm2[:])

                # ============ phase D: residual + LIF2 ============
                # t-outer emission: 8 independent (bp,hq) chains per step
                Pneg2 = {}
                for t in range(1 if PHASES >= 3 else 5, 5):
                    for bp in range(2):
                        p = (t - 1) * 2 + bp
                        iA = (t - 1) * 4 + bp * 2
                        for hq in range(NQ):
                            off = QL * hq
                            # wneg = Pneg - y2 on Pool, off the x path
                            wneg = hf.tile([128, QL], F32, tag="wvn", bufs=3)
                            if t == 1:
                                nc.gpsimd.tensor_tensor(
                                    wneg[:], zq[:], y2s[p][:, off:off + QL],
                                    AO.subtract)
                            else:
                                nc.gpsimd.tensor_tensor(
                                    wneg[:], Pneg2[bp, hq][:],
                                    y2s[p][:, off:off + QL], AO.subtract)
                            xs = hf.tile([128, QL], F32, tag="xs", bufs=4)
                            nc.sync.dma_start(
                                xs[:],
                                xin[iA:iA + 2, :, off:off + QL]
                                .rearrange("a p q -> (a p) q"))
                            # xsc = x*rsc2 in-place (Act)
                            nc.scalar.activation(xs[:], xs[:], AF.Copy,
                                                 scale=rsc2[:])
                            ot = hf.tile([128, QL], F16, tag="ot", bufs=3)
                            # s: (xsc - th) >= wneg  <=>  xsc + y2 + P >= th
                            nc.vector.scalar_tensor_tensor(
                                ot[:], xs[:], th2[:], wneg[:],
                                AO.subtract, AO.is_ge)
                            nc.scalar.dma_start(
                                outp[iA:iA + 2, :, off:off + QL]
                                .rearrange("a p q -> (a p) q"), ot[:])
                            if t < 4:
                                mng = hf.tile([128, QL], F16, tag="mng",
                                              bufs=2)
                                nc.scalar.activation(mng[:], ot[:],
                                                     AF.Identity,
                                                     bias=negone[:])
                                # u = q + gm2 = (xsc + gm2) - wneg, in-place
                                nc.vector.scalar_tensor_tensor(
                                    xs[:], xs[:], gm2[:], wneg[:],
                                    AO.add, AO.subtract)
                                Pn = hf.tile([128, QL], F32, tag="pp", bufs=8)
                                # (c2*u) * (s-1) = -c2*u*(1-s) = -P'
                                nc.vector.scalar_tensor_tensor(
                                    Pn[:], xs[:], c2, mng[:], AO.mult, AO.mult)
                                Pneg2[bp, hq] = Pn

    nc.compile()
    return nc, names


def _sigmoid(x):
    return 1.0 / (1.0 + np.exp(-float(x)))


def prepare(x, conv1_w, bn1_gamma, bn1_beta, lif1_w, conv2_w, bn2_gamma,
            bn2_beta, lif2_w):
    import ml_dtypes
    E4 = ml_dtypes.float8_e4m3

    x = np.ascontiguousarray(np.asarray(x, np.float32))
    conv1_w = np.asarray(conv1_w, np.float32)
    conv2_w = np.asarray(conv2_w, np.float32)

    a1 = _sigmoid(np.asarray(lif1_w).reshape(-1)[0])
    a2 = _sigmoid(np.asarray(lif2_w).reshape(-1)[0])

    key = (round(a1, 12), round(a2, 12))
    if key not in _prog_cache:
        _prog_cache[key] = _build(a1, a2)
    nc, names = _prog_cache[key]

    xh = x.astype(np.float16)
    xl = x - xh.astype(np.float32)

    # fp16 main planes: [xhi padded ; xhi shifted up one row]
    pa_np = np.zeros((T, B, 128, HP, HP), np.float16)
    pa_np[:, :, 0:64, 1:57, 1:57] = xh
    pa_np[:, :, 64:128, 0:56, 1:57] = xh
    # fp8 corr planes: [e4m3(x) ; e4m3(xl*2^11)], same (unshifted) layout
    pb_np = np.zeros((T, B, 128, HP, HP), E4)
    pb_np[:, :, 0:64, 1:57, 1:57] = x.astype(E4)
    pb_np[:, :, 64:128, 1:57, 1:57] = (xl * CSC).astype(E4)

    def pad_flat(arr):
        # [T,B,128,HP,HP] -> [T,B,128,PPP] with zero tail
        t_, b_, p_, _, _ = arr.shape
        out = np.zeros((t_, b_, p_, PPP), arr.dtype)
        out[..., :HP * HP] = arr.reshape(t_, b_, p_, HP * HP)
        return out

    pa_np = pad_flat(pa_np)
    pb_np = pad_flat(pb_np)

    w1h = conv1_w.astype(np.float16)
    w1l = conv1_w - w1h.astype(np.float32)
    w2h = conv2_w.astype(np.float16)
    w2l = conv2_w - w2h.astype(np.float32)

    def main_stack(wh):
        out = np.zeros((128, 6, 64), np.float16)
        for s, (di, dj) in enumerate(MAIN_SETS):
            if di == 0:
                out[0:64, s] = wh[:, :, 0, dj].T
                out[64:128, s] = wh[:, :, 1, dj].T
            else:
                out[0:64, s] = wh[:, :, 2, dj].T
        return out

    w1m_np = main_stack(w1h)

    w2m_np = main_stack(w2h)
    w2l_s = (w2l * CSC).astype(E4)
    w2c_np = np.zeros((128, 3, 2, 192), E4)
    for dj in range(3):
        w2c_np[0:64, dj, 0, 64:128] = w2l_s[:, :, 0, dj].astype(
            np.float32).T.astype(E4)
        w2c_np[64:128, dj, 0, 64:128] = w2l_s[:, :, 1, dj].astype(
            np.float32).T.astype(E4)
        w2c_np[0:64, dj, 1, 64:128] = w2l_s[:, :, 2, dj].astype(
            np.float32).T.astype(E4)

    # conv1 DR weights: k-tile block rows 0:64 multiply e4m3(x) -> W1l_s,
    # rows 64:128 multiply e4m3(xl*2^11) -> W1h8; active cols at 64:128.
    w1l_s = (w1l * CSC).astype(E4)
    w1h8 = conv1_w.astype(E4)
    w1c_np = np.zeros((128, 5, 2, 192), E4)
    dr1_taps = [((0, 0), (1, 0)), ((0, 1), (1, 1)), ((0, 2), (1, 2)),
                ((2, 0), (2, 2)), ((2, 1), None)]
    for i, (ta, tb) in enumerate(dr1_taps):
        for kk, tap in enumerate((ta, tb)):
            if tap is None:
                continue
            di, dj = tap
            w1c_np[0:64, i, kk, 64:128] = w1l_s[:, :, di, dj].astype(
                np.float32).T.astype(E4)
            w1c_np[64:128, i, kk, 64:128] = w1h8[:, :, di, dj].astype(
                np.float32).T.astype(E4)

    def dup(v):
        v = np.asarray(v, np.float32).reshape(64)
        return np.concatenate([v, v])

    cpar_np = np.zeros((128, 8), np.float32)
    cpar_np[:, 0] = dup(bn1_beta)
    cpar_np[:, 1] = dup(bn2_beta)
    cpar_np[:, 2] = 1.0 / dup(bn1_gamma)
    cpar_np[:, 3] = 1.0 / dup(bn2_gamma)
    rga1 = 1.0 / (a1 * dup(bn1_gamma))
    rga2 = 1.0 / (a2 * dup(bn2_gamma))
    cpar_np[:, 4] = rga1 * (1.0 - a1 * dup(bn1_beta))
    cpar_np[:, 5] = rga2 * (1.0 - a2 * dup(bn2_beta))

    in_maps = []
    for k in range(NCORES):
        sl = slice(4 * k, 4 * k + 4)
        pa_k = np.ascontiguousarray(pa_np[:, sl].reshape(NIMG, 128, PPP))
        pb_k = np.ascontiguousarray(pb_np[:, sl].reshape(NIMG, 128, PPP))
        xin_k = np.ascontiguousarray(x[:, sl].reshape(NIMG, 64, PIX))
        in_maps.append({
            names['pa']: pa_k,
            names['pb']: pb_k.view(np.uint8),
            names['xin']: xin_k,
            names['w1m']: w1m_np,
            names['w1c']: w1c_np.view(np.uint8),
            names['w2m']: w2m_np,
            names['w2c']: w2c_np.view(np.uint8),
            names['cpar']: cpar_np,
        })

    return nc, names, in_maps


def kernel(**inputs):
    from concourse.bass_utils import run_bass_kernel_spmd
    nc, names, in_maps = prepare(**inputs)
    res = run_bass_kernel_spmd(nc, in_maps, core_ids=list(range(NCORES)))
    global LAST_RES, LAST_NAMES
    LAST_RES, LAST_NAMES = res, names
    out = np.empty((T, B, C, H, W), np.float32)
    for k in range(NCORES):
        o = np.asarray(res.results[k][names['outp']], np.float32)
        out[:, 4 * k:4 * k + 4] = o.reshape(T, BL, C, H, W)
    return out


if __name__ == "__main__":
    rng = np.random.default_rng(0)
    xs = rng.standard_normal((T, B, C, H, W)).astype(np.float32)
    w1 = (rng.standard_normal((64, 64, 3, 3)) * 0.05).astype(np.float32)
    w2 = (rng.standard_normal((64, 64, 3, 3)) * 0.05).astype(np.float32)
    o = kernel(xs, w1, np.ones(64, np.float32), np.zeros(64, np.float32),
               np.zeros(1, np.float32), w2, np.ones(64, np.float32),
               np.zeros(64, np.float32), np.zeros(1, np.float32))
    print("ran:", o.shape, float(o.mean()))


# revision 37
# speedup vs baseline: 1.6562x; 1.0012x over previous
"""Trainium2 Bass kernel for nn_BasicBlock (spiking CNN block).

Sharding: data-parallel over batch B across 8 NeuronCores (4 batch x 4
timesteps = 16 images per core); BN batch stats via tiny AllReduce.

Per core (vs the 27-pass baseline this runs 16 pass-equivalents):
- conv1 main (W1hi . xhi, fp16): taps row-paired via planes laid out as
  [xhi ; xhi shifted one row] so K=128 contracts two taps -> 6 passes.
- conv1 corr (W1hi . xlo + W1lo . xhi): fp8e4 DoubleRow matmuls (0.5
  cyc/row) over planes [e4m3(x) ; e4m3(xlo*2^11)], two taps per
  instruction, M=128 with half-zero weight columns so each image's
  correction lands on its own partition half of ONE corr PSUM bank.
  Streams flat at the padded pitch (58); pad columns hold garbage that
  the evacuation never reads. ~2.5 pass-equivalents.
- conv2: same structure on spike planes [s1 ; s1 shifted] (s1 exact in
  fp16 AND fp8): hi 6 passes + W2lo fp8-DR corr 1.5 pass-equivalents.
- Evacuation: Act copy (main psum) -> strip, DVE stt strip += corr*2^-11
  (accum_out -> BN sums), Act square (accum_out -> BN sumsq).
- PLIF scans in "q-space" with negated-state fusion: s and the carried
  state are scalar_tensor_tensor ops; reset mask via Act (s-1).
- Residual+LIF2 spread across Act (x*rsc2, mask), Pool (adds), DVE
  (compares + state); spikes written out as fp16 and cast on host.
"""
import sys
sys.path.insert(0, '/opt/trn_rl_repo')

import numpy as np

T, B, C, H, W = 4, 32, 64, 56, 56
NCORES = 8
BL = B // NCORES            # 4 local batch samples
NIMG = T * BL               # 16 images per core
HP = W + 2                  # 58
PPP = HP * HP + 4           # padded plane + flat-stream overrun guard (3368)
PIX = H * W                 # 3136
NCH = 7                     # conv chunks per image (8 rows each)
CHW = 8 * W                 # 448 compact chunk
CFL = 8 * HP                # 464 flat chunk span
NPAIR = 8                   # image pairs per core
EPS = 1e-5
NG = float((T * B) * PIX)   # 401408
QL = 14 * W                 # LIF quarter-strip length (784)
NQ = 4
CSC = 2048.0                # 2^11 fp8 correction scale

# (di, dj) slice per fp16 main set: di=0 -> taps (0,dj)+(1,dj) paired via
# the shifted upper half; di=2 -> tap (2,dj) solo (upper weights zero).
MAIN_SETS = [(0, 0), (0, 1), (0, 2), (2, 0), (2, 1), (2, 2)]
# conv1 fp8-DR sets: (di, dj, dk) -> k-tile1 at (di,dj), k-tile2 at +dk
# covering tap pairs ((0,j),(1,j))x3, ((2,0),(2,2)) [stride 2; odd k-tile
# strides crash the ifmap fetcher], ((2,1), zero)
DR1_SETS = [(0, 0, HP), (0, 1, HP), (0, 2, HP), (2, 0, 2), (2, 1, 2)]
# conv2 fp8-DR sets: k-tiles 2 rows apart cover taps (0,j),(1,j),(2,j),x0
DR2_SETS = [(0, 0, 2 * HP), (0, 1, 2 * HP), (0, 2, 2 * HP)]

_prog_cache = {}
DBG = False
NO_CC = False
PHASES = 3
TRACE = False
LAST_RES = None
LAST_NAMES = None
LAST_EXEC_NS = None


def _build(alpha1, alpha2):
    import concourse.mybir as mybir
    import concourse.tile as tile
    from concourse.ap import AP
    from concourse import bacc

    F32 = mybir.dt.float32
    F16 = mybir.dt.float16
    F8 = mybir.dt.float8e4
    AO = mybir.AluOpType
    AF = mybir.ActivationFunctionType
    AX = mybir.AxisListType
    DRM = mybir.MatmulPerfMode.DoubleRow

    c1 = 1.0 - alpha1
    c2 = 1.0 - alpha2

    def sub_ap(base, extra_off, dims):
        b = base.copy()
        return AP(b.tensor, b.offset + extra_off,
                  [list(b.ap[0])] + [list(d) for d in dims])

    nc = bacc.Bacc(None, target_bir_lowering=False)
    names = {}

    with tile.TileContext(nc) as tc:
        with tc.tile_pool(name="dram", bufs=1, space="DRAM") as dram:
            pa = dram.tile([NIMG, 128, PPP], F16, kind="ExternalInput")
            pb = dram.tile([NIMG, 128, PPP], F8, kind="ExternalInput")
            xin = dram.tile([NIMG, 64, PIX], F32, kind="ExternalInput")
            w1m = dram.tile([128, 6, 64], F16, kind="ExternalInput")
            w1c = dram.tile([128, 5, 2, 192], F8, kind="ExternalInput")
            w2m = dram.tile([128, 6, 64], F16, kind="ExternalInput")
            w2c = dram.tile([128, 3, 2, 192], F8, kind="ExternalInput")
            cpar = dram.tile([128, 8], F32, kind="ExternalInput")
            outp = dram.tile([NIMG, 64, PIX], F16, kind="ExternalOutput")
            names.update(pa=pa.name, pb=pb.name, xin=xin.name, w1m=w1m.name,
                         w1c=w1c.name, w2m=w2m.name, w2c=w2c.name,
                         cpar=cpar.name, outp=outp.name)
            if DBG:
                y1d = dram.tile([NPAIR, 128, PIX], F32, kind="ExternalOutput")
                y2d = dram.tile([NPAIR, 128, PIX], F32, kind="ExternalOutput")
                s1d = dram.tile([NPAIR, 128, PIX], F32, kind="ExternalOutput")
                vecd = dram.tile([128, 8], F32, kind="ExternalOutput")
                names.update(y1d=y1d.name, y2d=y2d.name, s1d=s1d.name,
                             vecd=vecd.name)

            with tc.tile_pool(name="dramw", bufs=1, space="DRAM") as dramw, \
                 tc.tile_pool(name="wsb", bufs=1) as wsb, \
                 tc.tile_pool(name="ys", bufs=8) as yspool, \
                 tc.tile_pool(name="plane", bufs=4) as plpool, \
                 tc.tile_pool(name="hfp", bufs=2) as hf, \
                 tc.tile_pool(name="tiny", bufs=17) as tiny, \
                 tc.tile_pool(name="ps", bufs=7, space="PSUM") as ps:

                # ---- static parameter loads
                w1ms = wsb.tile([128, 6, 64], F16, tag="w1m")
                nc.sync.dma_start(w1ms[:], w1m[:])
                w1cs = wsb.tile([128, 5, 2, 192], F8, tag="w1c")
                nc.sync.dma_start(w1cs[:], w1c[:])
                w2ms = wsb.tile([128, 6, 64], F16, tag="w2m")
                nc.sync.dma_start(w2ms[:], w2m[:])
                w2cs = wsb.tile([128, 3, 2, 192], F8, tag="w2c")
                nc.sync.dma_start(w2cs[:], w2c[:])
                cpars = wsb.tile([128, 8], F32, tag="cpar")
                nc.sync.dma_start(cpars[:], cpar[:])
                negone = wsb.tile([128, 1], F32, tag="negone")
                nc.vector.memset(negone[:], -1.0)
                zq = wsb.tile([128, QL], F32, tag="zq")
                nc.vector.memset(zq[:], 0.0)
                epst = wsb.tile([128, 1], F32, tag="epst")
                nc.vector.memset(epst[:], EPS)
                sums1 = wsb.tile([128, 56], F32, tag="sums1")
                sums1q = wsb.tile([128, 56], F32, tag="sums1q")
                sums2 = wsb.tile([128, 56], F32, tag="sums2")
                sums2q = wsb.tile([128, 56], F32, tag="sums2q")
                if PHASES < 2:
                    nc.vector.memset(sums2[:], 0.0)
                    nc.vector.memset(sums2q[:], 0.0)

                def conv_img_pair(plA, plB, plA8, plB8, wm, wc, dr_sets,
                                  dst_strip, sums_t, sumsq_t, pcol):
                    """One image pair: per chunk, 6 fp16 tap-paired matmuls
                    per image into a main psum + fp8 DoubleRow correction
                    instructions (both images) into one flat corr psum, then
                    the 3-op evacuation."""
                    plAr = plA[:, 0:HP * HP].rearrange("p (r w) -> p r w", w=HP)
                    plBr = plB[:, 0:HP * HP].rearrange("p (r w) -> p r w", w=HP)
                    ndr = len(dr_sets)
                    for cth in range(NCH):
                        r0 = 8 * cth
                        pm = ps.tile([128, CHW], F32, tag="psm", bufs=5,
                                     name=f"psm{cth & 1}")
                        pc = ps.tile([128, CFL], F32, tag="psc", bufs=3,
                                     name=f"psc{cth & 1}")
                        for si, (di, dj) in enumerate(MAIN_SETS):
                            for j, plr in enumerate((plAr, plBr)):
                                rhs = plr[:, r0 + di:r0 + di + 8, dj:dj + W]
                                out = pm[64 * j:64 * (j + 1), :] \
                                    .rearrange("p (r w) -> p r w", r=8)
                                nc.tensor.matmul(
                                    out, wm[:, si, :], rhs,
                                    start=(si == 0), stop=(si == 5),
                                    tile_position=(0, 64 * j),
                                    skip_group_check=True)
                        idx = 0
                        for j, pl8 in enumerate((plA8, plB8)):
                            co = 64 * (1 - j)
                            for i, (di, dj, dk) in enumerate(dr_sets):
                                base = (r0 + di) * HP + dj
                                rhs = sub_ap(pl8[:], base, [(dk, 2), (1, CFL)])
                                nc.tensor.matmul(
                                    pc[:], wc[:, i, :, co:co + 128], rhs,
                                    start=(idx == 0), stop=(idx == 2 * ndr - 1),
                                    perf_mode=DRM, tile_position=(0, 0),
                                    skip_group_check=True)
                                idx += 1
                        # evacuation: copy main, add scaled corr, square
                        sl = dst_strip[:, CHW * cth:CHW * (cth + 1)]
                        slv = sl.rearrange("p (r w) -> p r w", w=W)
                        nc.scalar.activation(sl, pm[:], AF.Copy)
                        pcv = sub_ap(pc[:], 0, [(HP, 8), (1, W)])
                        nc.vector.scalar_tensor_tensor(
                            slv, pcv, 1.0 / CSC, slv, AO.mult, AO.add,
                            accum_out=sums_t[:, pcol * 7 + cth:pcol * 7 + cth + 1])
                        jk = hf.tile([128, CHW], F16, tag="jk", bufs=1)
                        nc.scalar.activation(
                            jk[:], sl, AF.Square,
                            accum_out=sumsq_t[:, pcol * 7 + cth:pcol * 7 + cth + 1])


                # ================= phase A: conv1 =================
                y1s = []
                for p in range(NPAIR):
                    tt_, bp = p // 2, p % 2
                    iA = tt_ * 4 + bp * 2
                    pls, pl8s_ = [], []
                    for j in range(2):
                        i = iA + j
                        ta = plpool.tile([128, PPP], F16, tag="plf16")
                        t8 = plpool.tile([128, PPP], F8, tag="plf8")
                        if p == 0:
                            hh = 30 * HP
                            nc.sync.dma_start(ta[:, 0:hh], pa[i, :, 0:hh])
                            nc.sync.dma_start(ta[:, hh:PPP], pa[i, :, hh:PPP])
                            nc.scalar.dma_start(t8[:, 0:hh], pb[i, :, 0:hh])
                            nc.scalar.dma_start(t8[:, hh:PPP], pb[i, :, hh:PPP])
                        else:
                            nc.sync.dma_start(ta[:], pa[i])
                            nc.scalar.dma_start(t8[:], pb[i])
                        pls.append(ta)
                        pl8s_.append(t8)
                    strip = yspool.tile([128, PIX], F32, tag="ys")
                    y1s.append(strip)
                    conv_img_pair(pls[0], pls[1], pl8s_[0], pl8s_[1],
                                  w1ms, w1cs, DR1_SETS, strip, sums1, sums1q, p)
                    if DBG:
                        nc.sync.dma_start(y1d[p], strip[:])

                # ---- stats1 allreduce
                cc1i = dramw.tile([128, 2], F32)
                cc1o = dramw.tile([128, 2], F32, addr_space="Shared")
                acc1 = tiny.tile([128, 2], F32, tag="acc")
                nc.vector.tensor_reduce(acc1[:, 0:1], sums1[:], AX.X, AO.add)
                nc.vector.tensor_reduce(acc1[:, 1:2], sums1q[:], AX.X, AO.add)
                nc.sync.dma_start(cc1i[:], acc1[:])
                if NO_CC:
                    nc.sync.dma_start(cc1o[:], cc1i[:])
                else:
                    nc.gpsimd.collective_compute(
                        "AllReduce", AO.add, ins=[cc1i[:]], outs=[cc1o[:]],
                        replica_groups=[list(range(NCORES))])

                def stats_block(cco, beta, rgam, k1):
                    g = tiny.tile([128, 2], F32, tag="acc")
                    nc.sync.dma_start(g[:], cco[:])
                    gr = tiny.tile([128, 2], F32, tag="acc")
                    nc.scalar.dma_start(gr[0:64, :], cco[64:128, :])
                    nc.scalar.dma_start(gr[64:128, :], cco[0:64, :])
                    tot = tiny.tile([128, 2], F32, tag="acc")
                    nc.vector.tensor_tensor(tot[:], g[:], gr[:], AO.add)
                    mean = tiny.tile([128, 1], F32, tag="t1")
                    nc.vector.tensor_scalar(mean[:], tot[:, 0:1], 1.0 / NG,
                                            None, AO.mult)
                    msq = tiny.tile([128, 1], F32, tag="t1")
                    nc.vector.tensor_scalar(msq[:], tot[:, 1:2], 1.0 / NG,
                                            None, AO.mult)
                    m2 = tiny.tile([128, 1], F32, tag="t1")
                    nc.vector.scalar_tensor_tensor(m2[:], mean[:], 1.0, mean[:],
                                                   AO.bypass, AO.mult)
                    var = tiny.tile([128, 1], F32, tag="t1")
                    nc.vector.tensor_tensor(var[:], msq[:], m2[:], AO.subtract)
                    std = tiny.tile([128, 1], F32, tag="t1")
                    nc.scalar.activation(std[:], var[:], AF.Sqrt, bias=epst[:])
                    rscv = tiny.tile([128, 1], F32, tag="t1")
                    nc.vector.tensor_tensor(rscv[:], std[:], rgam, AO.mult)
                    gamv = tiny.tile([128, 1], F32, tag="t1")
                    # gm = beta*rsc - mean
                    nc.vector.scalar_tensor_tensor(gamv[:], rscv[:], beta,
                                                   mean[:], AO.mult,
                                                   AO.subtract)
                    th = tiny.tile([128, 1], F32, tag="t1")
                    # th = std*K1 + mean, K1 = rga*(1 - alpha*beta)
                    nc.vector.scalar_tensor_tensor(th[:], std[:], k1,
                                                   mean[:], AO.mult, AO.add)
                    return th, gamv, rscv

                th1, gm1, _rsc1 = stats_block(
                    cc1o, cpars[:, 0:1], cpars[:, 2:3], cpars[:, 4:5])
                # wv1 bias: c1*gm1 so wv = c1*(q + gm1); Pneg = wv*(s-1) = -P
                gm1c1 = tiny.tile([128, 1], F32, tag="t1")
                nc.vector.tensor_scalar(gm1c1[:], gm1[:], c1, None, AO.mult)
                if DBG:
                    nc.sync.dma_start(vecd[:, 0:1], th1[:])
                    nc.sync.dma_start(vecd[:, 1:2], gm1[:])
                    nc.sync.dma_start(vecd[:, 4:5], acc1[:, 0:1])
                    nc.sync.dma_start(vecd[:, 5:6], acc1[:, 1:2])

                # ============ phase B + C: LIF1 + conv2 ============
                y2s = [None] * NPAIR
                for bp in range(2 if PHASES >= 2 else 0):
                    Pneg = [None] * NQ
                    for t in range(1, 5):
                        p = (t - 1) * 2 + bp
                        s1tq = []
                        for hq in range(NQ):
                            off = QL * hq
                            ysl = y1s[p][:, off:off + QL]
                            s1t8 = hf.tile([128, QL], F8, tag="s1t8",
                                           bufs=3)
                            if t == 1:
                                nc.vector.tensor_scalar(s1t8[:], ysl, th1[:],
                                                        None, AO.is_ge)
                            else:
                                # s = (y1 - th) >= Pneg  <=>  y1 + P >= th
                                nc.vector.scalar_tensor_tensor(
                                    s1t8[:], ysl, th1[:], Pneg[hq][:],
                                    AO.subtract, AO.is_ge)
                            s1tq.append(s1t8)
                            if DBG:
                                nc.gpsimd.dma_start(
                                    s1d[p, :, off:off + QL], s1t8[:])
                            if t < 4:
                                mng = hf.tile([128, QL], F16, tag="mng", bufs=2)
                                nc.scalar.activation(mng[:], s1t8[:],
                                                     AF.Identity,
                                                     bias=negone[:])
                                # v = q + gm1 = (y1 + gm1) - Pneg
                                v = hf.tile([128, QL], F32, tag="wvn", bufs=3)
                                if t == 1:
                                    nc.vector.tensor_scalar(
                                        v[:], ysl, gm1[:], None, AO.add)
                                else:
                                    nc.vector.scalar_tensor_tensor(
                                        v[:], ysl, gm1[:], Pneg[hq][:],
                                        AO.add, AO.subtract)
                                Pn = hf.tile([128, QL], F32, tag="pp", bufs=8)
                                # (c1*v) * (s-1) = -c1*v*(1-s) = -P'
                                nc.vector.scalar_tensor_tensor(
                                    Pn[:], v[:], c1, mng[:], AO.mult, AO.mult)
                                Pneg[hq] = Pn
                        # fp8 spike planes [s1 ; s1 shifted one row] serve
                        # both the f16-weight main passes and the fp8 DR corr
                        s8_pair = []
                        for j in range(2):
                            s8 = plpool.tile([128, PPP], F8, tag="plf8")
                            s8r = s8[:, 0:HP * HP].rearrange(
                                "p (r w) -> p r w", w=HP)
                            for hq in range(NQ):
                                s1t8 = s1tq[hq]
                                src8 = s1t8[64 * j:64 * (j + 1), :] \
                                    .rearrange("p (r w) -> p r w", w=W)
                                rl = 1 + 14 * hq
                                ru = 14 * hq
                                nc.sync.dma_start(
                                    s8r[0:64, rl:rl + 14, 1:1 + W], src8)
                                nc.scalar.dma_start(
                                    s8r[64:128, ru:ru + 14, 1:1 + W], src8)
                            s8_pair.append(s8)
                        strip2 = yspool.tile([128, PIX], F32, tag="ys")
                        y2s[p] = strip2
                        conv_img_pair(s8_pair[0], s8_pair[1], s8_pair[0],
                                      s8_pair[1], w2ms, w2cs, DR2_SETS,
                                      strip2, sums2, sums2q, p)
                        if DBG:
                            nc.sync.dma_start(y2d[p], strip2[:])

                # ---- stats2 allreduce
                cc2i = dramw.tile([128, 2], F32)
                cc2o = dramw.tile([128, 2], F32, addr_space="Shared")
                acc2 = tiny.tile([128, 2], F32, tag="acc")
                nc.vector.tensor_reduce(acc2[:, 0:1], sums2[:], AX.X, AO.add)
                nc.vector.tensor_reduce(acc2[:, 1:2], sums2q[:], AX.X, AO.add)
                nc.sync.dma_start(cc2i[:], acc2[:])
                if NO_CC:
                    nc.sync.dma_start(cc2o[:], cc2i[:])
                else:
                    nc.gpsimd.collective_compute(
                        "AllReduce", AO.add, ins=[cc2i[:]], outs=[cc2o[:]],
                        replica_groups=[list(range(NCORES))])
                th2, gm2, rsc2 = stats_block(
                    cc2o, cpars[:, 1:2], cpars[:, 3:4], cpars[:, 5:6])
                if DBG:
                    nc.sync.dma_start(vecd[:, 2:3], th2[:])
                    nc.sync.dma_start(vecd[:, 3:4], gm2[:])

                # ============ phase D: residual + LIF2 ============
                # t-outer emission: 8 independent (bp,hq) chains per step
                Pneg2 = {}
                for t in range(1 if PHASES >= 3 else 5, 5):
                    for bp in range(2):
                        p = (t - 1) * 2 + bp
                        iA = (t - 1) * 4 + bp * 2
                        for hq in range(NQ):
                            off = QL * hq
                            # wneg = Pneg - y2 on Pool, off the x path
                            wneg = hf.tile([128, QL], F32, tag="wvn", bufs=3)
                            if t == 1:
                                nc.gpsimd.tensor_tensor(
                                    wneg[:], zq[:], y2s[p][:, off:off + QL],
                                    AO.subtract)
                            else:
                                nc.gpsimd.tensor_tensor(
                                    wneg[:], Pneg2[bp, hq][:],
                                    y2s[p][:, off:off + QL], AO.subtract)
                            xs = hf.tile([128, QL], F32, tag="xs", bufs=4)
                            nc.sync.dma_start(
                                xs[:],
                                xin[iA:iA + 2, :, off:off + QL]
                                .rearrange("a p q -> (a p) q"))
                            # xsc = x*rsc2 in-place (Act)
                            nc.scalar.activation(xs[:], xs[:], AF.Copy,
                                                 scale=rsc2[:])
                            ot = hf.tile([128, QL], F16, tag="ot", bufs=3)
                            # s: (xsc - th) >= wneg  <=>  xsc + y2 + P >= th
                            nc.vector.scalar_tensor_tensor(
                                ot[:], xs[:], th2[:], wneg[:],
                                AO.subtract, AO.is_ge)
                            nc.scalar.dma_start(
                                outp[iA:iA + 2, :, off:off + QL]
                                .rearrange("a p q -> (a p) q"), ot[:])
                            if t < 4:
                                mng = hf.tile([128, QL], F16, tag="mng",
                                              bufs=2)
                                nc.scalar.activation(mng[:], ot[:],
                                                     AF.Identity,
                                                     bias=negone[:])
                                # u = q + gm2 = (xsc + gm2) - wneg, in-place
                                nc.vector.scalar_tensor_tensor(
                                    xs[:], xs[:], gm2[:], wneg[:],
                                    AO.add, AO.subtract)
                                Pn = hf.tile([128, QL], F32, tag="pp", bufs=8)
                                # (c2*u) * (s-1) = -c2*u*(1-s) = -P'
                                nc.vector.scalar_tensor_tensor(
                                    Pn[:], xs[:], c2, mng[:], AO.mult, AO.mult)
                                Pneg2[bp, hq] = Pn

    nc.compile()
    return nc, names


def _sigmoid(x):
    return 1.0 / (1.0 + np.exp(-float(x)))


def prepare(x, conv1_w, bn1_gamma, bn1_beta, lif1_w, conv2_w, bn2_gamma,
            bn2_beta, lif2_w):
    import ml_dtypes
    E4 = ml_dtypes.float8_e4m3

    x = np.ascontiguousarray(np.asarray(x, np.float32))
    conv1_w = np.asarray(conv1_w, np.float32)
    conv2_w = np.asarray(conv2_w, np.float32)

    a1 = _sigmoid(np.asarray(lif1_w).reshape(-1)[0])
    a2 = _sigmoid(np.asarray(lif2_w).reshape(-1)[0])

    key = (round(a1, 12), round(a2, 12))
    if key not in _prog_cache:
        _prog_cache[key] = _build(a1, a2)
    nc, names = _prog_cache[key]

    xh = x.astype(np.float16)
    xl = x - xh.astype(np.float32)

    # fp16 main planes: [xhi padded ; xhi shifted up one row]
    pa_np = np.zeros((T, B, 128, HP, HP), np.float16)
    pa_np[:, :, 0:64, 1:57, 1:57] = xh
    pa_np[:, :, 64:128, 0:56, 1:57] = xh
    # fp8 corr planes: [e4m3(x) ; e4m3(xl*2^11)], same (unshifted) layout
    pb_np = np.zeros((T, B, 128, HP, HP), E4)
    pb_np[:, :, 0:64, 1:57, 1:57] = x.astype(E4)
    pb_np[:, :, 64:128, 1:57, 1:57] = (xl * CSC).astype(E4)

    def pad_flat(arr):
        # [T,B,128,HP,HP] -> [T,B,128,PPP] with zero tail
        t_, b_, p_, _, _ = arr.shape
        out = np.zeros((t_, b_, p_, PPP), arr.dtype)
        out[..., :HP * HP] = arr.reshape(t_, b_, p_, HP * HP)
        return out

    pa_np = pad_flat(pa_np)
    pb_np = pad_flat(pb_np)

    w1h = conv1_w.astype(np.float16)
    w1l = conv1_w - w1h.astype(np.float32)
    w2h = conv2_w.astype(np.float16)
    w2l = conv2_w - w2h.astype(np.float32)

    def main_stack(wh):
        out = np.zeros((128, 6, 64), np.float16)
        for s, (di, dj) in enumerate(MAIN_SETS):
            if di == 0:
                out[0:64, s] = wh[:, :, 0, dj].T
                out[64:128, s] = wh[:, :, 1, dj].T
            else:
                out[0:64, s] = wh[:, :, 2, dj].T
        return out

    w1m_np = main_stack(w1h)

    w2m_np = main_stack(w2h)
    w2l_s = (w2l * CSC).astype(E4)
    w2c_np = np.zeros((128, 3, 2, 192), E4)
    for dj in range(3):
        w2c_np[0:64, dj, 0, 64:128] = w2l_s[:, :, 0, dj].astype(
            np.float32).T.astype(E4)
        w2c_np[64:128, dj, 0, 64:128] = w2l_s[:, :, 1, dj].astype(
            np.float32).T.astype(E4)
        w2c_np[0:64, dj, 1, 64:128] = w2l_s[:, :, 2, dj].astype(
            np.float32).T.astype(E4)

    # conv1 DR weights: k-tile block rows 0:64 multiply e4m3(x) -> W1l_s,
    # rows 64:128 multiply e4m3(xl*2^11) -> W1h8; active cols at 64:128.
    w1l_s = (w1l * CSC).astype(E4)
    w1h8 = conv1_w.astype(E4)
    w1c_np = np.zeros((128, 5, 2, 192), E4)
    dr1_taps = [((0, 0), (1, 0)), ((0, 1), (1, 1)), ((0, 2), (1, 2)),
                ((2, 0), (2, 2)), ((2, 1), None)]
    for i, (ta, tb) in enumerate(dr1_taps):
        for kk, tap in enumerate((ta, tb)):
            if tap is None:
                continue
            di, dj = tap
            w1c_np[0:64, i, kk, 64:128] = w1l_s[:, :, di, dj].astype(
                np.float32).T.astype(E4)
            w1c_np[64:128, i, kk, 64:128] = w1h8[:, :, di, dj].astype(
                np.float32).T.astype(E4)

    def dup(v):
        v = np.asarray(v, np.float32).reshape(64)
        return np.concatenate([v, v])

    cpar_np = np.zeros((128, 8), np.float32)
    cpar_np[:, 0] = dup(bn1_beta)
    cpar_np[:, 1] = dup(bn2_beta)
    cpar_np[:, 2] = 1.0 / dup(bn1_gamma)
    cpar_np[:, 3] = 1.0 / dup(bn2_gamma)
    rga1 = 1.0 / (a1 * dup(bn1_gamma))
    rga2 = 1.0 / (a2 * dup(bn2_gamma))
    cpar_np[:, 4] = rga1 * (1.0 - a1 * dup(bn1_beta))
    cpar_np[:, 5] = rga2 * (1.0 - a2 * dup(bn2_beta))

    in_maps = []
    for k in range(NCORES):
        sl = slice(4 * k, 4 * k + 4)
        pa_k = np.ascontiguousarray(pa_np[:, sl].reshape(NIMG, 128, PPP))
        pb_k = np.ascontiguousarray(pb_np[:, sl].reshape(NIMG, 128, PPP))
        xin_k = np.ascontiguousarray(x[:, sl].reshape(NIMG, 64, PIX))
        in_maps.append({
            names['pa']: pa_k,
            names['pb']: pb_k.view(np.uint8),
            names['xin']: xin_k,
            names['w1m']: w1m_np,
            names['w1c']: w1c_np.view(np.uint8),
            names['w2m']: w2m_np,
            names['w2c']: w2c_np.view(np.uint8),
            names['cpar']: cpar_np,
        })

    return nc, names, in_maps


def kernel(**inputs):
    from concourse.bass_utils import run_bass_kernel_spmd
    nc, names, in_maps = prepare(**inputs)
    res = run_bass_kernel_spmd(nc, in_maps, core_ids=list(range(NCORES)))
    global LAST_RES, LAST_NAMES
    LAST_RES, LAST_NAMES = res, names
    out = np.empty((T, B, C, H, W), np.float32)
    for k in range(NCORES):
        o = np.asarray(res.results[k][names['outp']], np.float32)
        out[:, 4 * k:4 * k + 4] = o.reshape(T, BL, C, H, W)
    return out


if __name__ == "__main__":
    rng = np.random.default_rng(0)
    xs = rng.standard_normal((T, B, C, H, W)).astype(np.float32)
    w1 = (rng.standard_normal((64, 64, 3, 3)) * 0.05).astype(np.float32)
    w2 = (rng.standard_normal((64, 64, 3, 3)) * 0.05).astype(np.float32)
    o = kernel(xs, w1, np.ones(64, np.float32), np.zeros(64, np.float32),
               np.zeros(1, np.float32), w2, np.ones(64, np.float32),
               np.zeros(64, np.float32), np.zeros(1, np.float32))
    print("ran:", o.shape, float(o.mean()))
